# revision 66
# baseline (speedup 1.0000x reference)
"""Trainium2 Bass kernel for nn_CopulaDecoder.  (HW: ~518us, rel err 3.7e-3)

Data-parallel over batch: core b computes batch element b end-to-end.
All activations live transposed (features on partitions, tokens on free dim).
The neighbor-gather softmax is reformulated as a dense count-matrix softmax:
  softmax over the 64 gathered scores == (C * exp(scale*S)) normalized, where
  C[p,v] = sum_n 1[neighbor_index[p,n]==v] * exp(-scale*attn_mask[p,n]).
Scores are small (|scale*S| < ~4 for this model family), so no max-shift.

Bias algebra: the key-MLP output bias kb3 shifts every score for a given
(p, head) by the same constant q.kb3 -> cancels in softmax -> dropped.
The value-MLP output bias vb3 shifts the attention output by vb3 (softmax
weights sum to 1) -> folded into the residual add before LN1.  Target
one-hots for the final NLL are host-precomputed; logits are O(10) so the
log-softmax needs no max-shift.  All reciprocals (LN rstd, softmax denom)
are exp(-ln x) / exp(-0.5 ln(x+eps)) on ACT: Ln+Exp live in one activation
table set (natural_log_exp), so the kernel never reloads ACT tables and
avoids the 8-cycle/elem DVE divide.

Precision: fp8-e4m3 DoubleRow (K=256/pass) for the two big KV-MLP matmuls,
bf16 for mm3 + attention, fp16 single for the small matmuls (ds/ff/decoder)
with f32 per-partition biases applied in the epilogue ops (no bias_mm
rank-1s, no hi/lo splits), fp32 accumulate, fp32 elementwise.

Scheduling: one software pipeline across the whole net.  Emission order ==
engine queue order (in-order engines), so overlap is achieved by
interleaving emission subject to producer-before-consumer order:
  k0[0:4] -> [attn0-q0 (av_lag) || v0] -> [attn0 || k0/v0 tails, k1] ->
  [ff0 || v1[0:4]] -> [attn1 || v1[4:8]] -> ff1 -> decoder (pipelined
  logits).  DMA hoists (count matrix in 16 pieces, weights) are spread
  across the head chains so no transfer ever blocks chain weights.
attention defers AV emission by av_lag steps (deep ce ring) so its
QK/exp/ce can overlap chains that produce the values; the per-quad
denominator pipeline (psA evacuate -> recip -> broadcast -> residual) is
deferred into the next quad's steps to avoid head-of-line PE stalls.
The true_u rank-1 (K=1) matmuls are row-packed pairs (rows 64fc+32j) that
run concurrently in the PE sub-arrays; QK is 4-way row-packed with all
four matmuls emitted back-to-back; AV/mm3-k use column packing.
PSUM: one [128,1024] ring (bufs=3) shared by chains/scores/misc + a
[128,512] AV ring (bufs=2).
"""
import os

import numpy as np
import ml_dtypes

B, S, T = 8, 32, 64
V = S * T
P = 512
N = 2 * S
I = 256
H, AD = 8, 32
D = H * AD
M = 256
L = 2
R = 128
SCALE = float(AD) ** -0.5

BF = ml_dtypes.bfloat16

_BUILT = {}


# ---------------------------------------------------------------------------
# walrus wait-slot workaround (inlined; see dev notes): Tile attaches >1
# semaphore wait to one instruction; many ISA encodings have a single wait
# slot.  Peel excess waits onto injected same-engine InstNoOps.
# ---------------------------------------------------------------------------
def _install_waitfix():
    import bass_rust
    import concourse.mybir as mybir
    import concourse.tile as tile_mod

    if getattr(tile_mod.TileContext, "_waitfix_installed", False):
        return
    limits = {"InstDrain": 1000, "InstEventSemaphore": 1000, "InstCall": 1000,
              "InstISA": 0}
    counter = [0]
    orig_add = tile_mod.TileContext._add_instruction

    def patched_add(self, inst):
        si = inst.sync_info
        if si is not None:
            limit = limits.get(type(inst).__name__, 1)
            waits = list(si.on_wait)
            if len(waits) > limit:
                keep = waits[-limit:] if limit else []
                excess = waits[:-limit] if limit else waits
                while excess:
                    chunk, excess = excess[:1], excess[1:]
                    counter[0] += 1
                    nop = bass_rust.InstNoOp(
                        name=f"waitsplit-{counter[0]}", ins=[], outs=[])
                    nop.engine = inst.engine
                    nop.sync_info = mybir.SyncInfo(on_wait=chunk, on_update=[])
                    orig_add(self, nop)
                inst.sync_info = mybir.SyncInfo(
                    on_wait=keep, on_update=list(si.on_update))
        orig_add(self, inst)

    def patched_drain_and_barrier(self, tick_clock, wait_clock):
        from concourse.tile import ScopedClock

        drain_inst = self.nc.sync.drain()
        wait_clock.add_sem_waits(
            drain_inst.ins, ScopedClock({None: tick_clock.global_clock}))
        si = drain_inst.ins.sync_info
        if si is not None and len(si.on_wait) > 1:
            waits = list(si.on_wait)
            drain_inst.ins.sync_info = mybir.SyncInfo(
                on_wait=waits[:1], on_update=list(si.on_update))
            rest = waits[1:]
            while rest:
                chunk, rest = rest[:1], rest[1:]
                nop = self.nc.sync.nop()
                nop.ins.sync_info = mybir.SyncInfo(on_wait=chunk, on_update=[])
        self.nc.all_engine_barrier()
        assert self.sems is not None
        popped = self.nc._tile_sem_poison_stack.pop()
        assert popped is self._sem_poison
        self.nc.clear_and_free_semaphores(list(self.sems.allocated().values()))
        self.nc.all_engine_barrier()

    try:
        import concourse.tile_utils as tile_utils
        tile_utils.max_sbuf_usage = 204 * 1024
    except Exception:
        pass
    tile_mod.TileContext._add_instruction = patched_add
    tile_mod.TileContext._drain_and_barrier = patched_drain_and_barrier
    tile_mod.TileContext._waitfix_installed = True


def _build():
    """Emit the single-core Bass program (SPMD across 8 cores)."""
    import concourse.bass as bass
    import concourse.mybir as mybir
    import concourse.tile as tile

    _install_waitfix()

    F32 = mybir.dt.float32
    BF16 = mybir.dt.bfloat16
    FP16 = mybir.dt.float16
    FP8 = mybir.dt.float8e4
    DR = mybir.MatmulPerfMode.DoubleRow
    AF = mybir.ActivationFunctionType
    ALU = mybir.AluOpType

    nc = bass.Bass()

    def din(name, shape, dt=BF16):
        return nc.dram_tensor(name, list(shape), dt, kind="ExternalInput")

    # --- DRAM inputs -------------------------------------------------------
    xtd = din("xtd", [128, 2, V], mybir.dt.float8e4)  # merged.T rows 0:256
    xt2 = din("xt2", [4, V])              # row 256 (true_u)            (bf16)
    ctm = din("ctm", [V, P])              # count matrix transposed     (bf16)
    cur16d = din("cur16d", [I, P], FP16)  # cur.T                       (fp16)
    updc = din("updc", [P, 1], F32)       # true_u at pred points       (f32)

    kvw = {}
    for pre in ("k", "v"):
        kvw[pre + "12"] = din(pre + "w12", [L, H, 128, 4, M],
                              mybir.dt.float8e4)
        kvw[pre + "1c"] = din(pre + "w1c", [L, H, 4, M])
        kvw[pre + "3"] = din(pre + "w3", [L, H, 128, 2 * AD])
        kvw[pre + "b"] = din(pre + "bb", [L, H, 128, 4], mybir.dt.float32)
    vb3qd = din("vb3qd", [L, 2, 128, 1], F32)

    dsw16d = din("dsw16d", [I, D], FP16)
    dsbd = din("dsbd", [2, 128, 1], F32)
    ffw1d = din("ffw1d", [L, D, D], FP16)
    ffw2d = din("ffw2d", [L, D, D], FP16)
    ffb1d = din("ffb1d", [L, 2, 128, 1], F32)
    ffb2d = din("ffb2d", [L, 2, 128, 1], F32)
    ln1gd = din("ln1gd", [L, 2, 128, 1], F32)
    ln1bd = din("ln1bd", [L, 2, 128, 1], F32)
    ln2gd = din("ln2gd", [L, 2, 128, 1], F32)
    ln2bd = din("ln2bd", [L, 2, 128, 1], F32)
    dew1d = din("dew1d", [D, M], FP16)
    dew2d = din("dew2d", [M, M], FP16)
    dew3d = din("dew3d", [M, R], FP16)
    deb1d = din("deb1d", [2, 128, 1], F32)
    deb2d = din("deb2d", [2, 128, 1], F32)
    deb3d = din("deb3d", [1, R], F32)

    oh8d = din("oh8d", [4, 128])          # onehot head->feat-rows (bf16)
    ohtd = din("ohtd", [128, 4, R], F32)  # onehot target classes per pred

    out_d = nc.dram_tensor("out", [1, 1], F32, kind="ExternalOutput")

    with tile.TileContext(nc) as tc:
        with (
            tc.tile_pool(name="const", bufs=1) as cpool,
            tc.tile_pool(name="resident", bufs=1) as rpool,
            tc.tile_pool(name="wts", bufs=2) as wpool,
            tc.tile_pool(name="work", bufs=1) as kpool,
            tc.tile_pool(name="psum", bufs=1, space="PSUM") as pp,
        ):
            # --- constants / resident tensors ---------------------------
            ones_c128b = cpool.tile([128, 1], BF16, name="ones_c128b")
            nc.vector.memset(ones_c128b[:], 1.0)
            ones_c128f = cpool.tile([128, 1], F32, name="ones_c128f")
            nc.vector.memset(ones_c128f[:], 1.0)
            ones_rbf = cpool.tile([1, 128], F32, name="ones_rbf")
            nc.vector.memset(ones_rbf[:], 1.0)
            ones_rb16 = cpool.tile([1, 128], BF16, name="ones_rb16")
            nc.vector.memset(ones_rb16[:], 1.0)
            eps_t = cpool.tile([1, 1], F32, name="eps_t")
            nc.vector.memset(eps_t[:], 1e-5)
            nlogr_t = cpool.tile([1, 1], F32, name="nlogr_t")
            nc.vector.memset(nlogr_t[:], -float(P) * float(np.log(R)))
            oh4 = cpool.tile([4, 128], BF16, name="oh4")
            nc.sync.dma_start(oh4[:], oh8d[:])

            # u replicated at partitions 0/32/64/96 for 4-way row-packed
            # K=1 matmuls (true_u rank-1 term of mm1); issued before the
            # bulk xt transfer (first chain needs u4 + xt half 0 only)
            u4 = rpool.tile([128, V], BF16, name="u4")
            nc.sync.dma_start(
                u4.rearrange("(r c) v -> r c v", c=32)[:, 0:1, :], xt2[:])
            xt = rpool.tile([128, 2, V], FP8, name="xt")
            nc.sync.dma_start(xt[:, :, 0:1024], xtd[:, :, 0:1024])
            nc.sync.dma_start(xt[:, :, 1024:2048], xtd[:, :, 1024:2048])

            ct = rpool.tile([128, 16, P], BF16, name="ct")

            cur16 = [kpool.tile([128, P], FP16, tag=f"cur{q}", bufs=1,
                                name=f"cur{q}") for q in range(2)]

            # hoisted ff + decoder weights (resident; off the startup path)
            fw1 = [[cpool.tile([128, D], FP16, name=f"fw1_{l}{kc}")
                    for kc in range(2)] for l in range(L)]
            fw2 = [[cpool.tile([128, D], FP16, name=f"fw2_{l}{kc}")
                    for kc in range(2)] for l in range(L)]
            ffb = {}
            for l in range(L):
                for nm, _src in (("b1", ffb1d), ("b2", ffb2d)):
                    for fc in range(2):
                        ffb[(nm, l, fc)] = cpool.tile(
                            [128, 1], F32, name=f"ff{nm}{l}{fc}")
            lnw = {}
            for nm in ("ln1g", "ln1b", "ln2g", "ln2b"):
                for l in range(L):
                    for q in range(2):
                        lnw[(nm, l, q)] = cpool.tile(
                            [128, 1], F32, name=f"{nm}{l}{q}")
            vb3t = {}
            for l in range(L):
                for q in range(2):
                    vb3t[(l, q)] = cpool.tile([128, 1], F32,
                                              name=f"vb3t{l}{q}")
            dw1 = [cpool.tile([128, M], FP16, name=f"dw1_{kc}")
                   for kc in range(2)]
            dw2 = [cpool.tile([128, M], FP16, name=f"dw2_{kc}")
                   for kc in range(2)]
            dw3 = [cpool.tile([128, R], FP16, name=f"dw3_{kc}")
                   for kc in range(2)]
            deb1 = [cpool.tile([128, 1], F32, name=f"deb1_{fc}")
                    for fc in range(2)]
            deb2 = [cpool.tile([128, 1], F32, name=f"deb2_{fc}")
                    for fc in range(2)]
            db3f = cpool.tile([1, R], F32, name="db3f")

            def ct_piece(c0, c1):
                # ct[p, c, q] = ctm[c*128 + p, q]; contiguous 128-row blocks
                for c in range(c0, c1):
                    nc.sync.dma_start(ct[:, c, :],
                                      ctm[128 * c:128 * (c + 1), :])

            def hoist_ln():
                lnsrc = {"ln1g": ln1gd, "ln1b": ln1bd,
                         "ln2g": ln2gd, "ln2b": ln2bd}
                for nm in ("ln1g", "ln1b", "ln2g", "ln2b"):
                    for l in range(L):
                        for q in range(2):
                            nc.sync.dma_start(
                                lnw[(nm, l, q)][:], lnsrc[nm][l, q, :, :])

            def hoist_ffb():
                for l in range(L):
                    for fc in range(2):
                        nc.sync.dma_start(ffb[("b1", l, fc)][:],
                                          ffb1d[l, fc, :, :])
                        nc.sync.dma_start(ffb[("b2", l, fc)][:],
                                          ffb2d[l, fc, :, :])
                        nc.sync.dma_start(vb3t[(l, fc)][:],
                                          vb3qd[l, fc, :, :])

            def hoist_fw(l):
                for kc in range(2):
                    sl = slice(128 * kc, 128 * (kc + 1))
                    nc.sync.dma_start(fw1[l][kc][:], ffw1d[l, sl, :])
                    nc.sync.dma_start(fw2[l][kc][:], ffw2d[l, sl, :])

            def hoist_dw():
                for kc in range(2):
                    sl = slice(128 * kc, 128 * (kc + 1))
                    nc.sync.dma_start(dw1[kc][:], dew1d[sl, :])
                    nc.sync.dma_start(dw2[kc][:], dew2d[sl, :])
                    nc.sync.dma_start(dw3[kc][:], dew3d[sl, :])
                for fc in range(2):
                    nc.sync.dma_start(deb1[fc][:], deb1d[fc, :, :])
                    nc.sync.dma_start(deb2[fc][:], deb2d[fc, :, :])
                nc.sync.dma_start(db3f[:], deb3d[:])

            hoists = [
                lambda: ct_piece(0, 2),
                lambda: (ct_piece(2, 4), hoist_ln()),
                lambda: (ct_piece(4, 6), hoist_ffb()),
                lambda: (ct_piece(6, 8), hoist_fw(0)),
                lambda: (ct_piece(8, 10), hoist_fw(1)),
                lambda: (ct_piece(10, 12), hoist_dw()),
                lambda: ct_piece(12, 16),
            ]

            # keys (transposed, per (l, quad)) and vals (+ones, per (l,h))
            kt = [[rpool.tile([128, V], BF16, name=f"kt{l}{q}")
                   for q in range(2)] for l in range(L)]
            vals = [[rpool.tile([128, 16, AD + 1], BF16, name=f"vals{l}{h}")
                     for h in range(H)] for l in range(L)]

            def big_tile(nm):
                return pp.tile([128, 1024], mybir.dt.float32, tag="big",
                               name=nm, bufs=3, uniquify=True)

            def psA_tile(nm):
                return pp.tile([128, 512], mybir.dt.float32, tag="psA",
                               name=nm, bufs=2, uniquify=True)

            # elementwise engine assignment: 3/5 ACT, 2/5 DVE (DVE also
            # carries the count-multiply + LN chains)
            ew_state = [0]
            ew_pat = [(1, 0)]

            def ew_next():
                pat = ew_pat[0]
                s = ew_state[0]
                ew_state[0] = (s + 1) % len(pat)
                return pat[s % len(pat)]

            def relu_out(dst, src_ps, bias):
                """dst = relu(src_ps + bias), alternating ACT/DVE."""
                if ew_next():
                    nc.scalar.activation(dst, src_ps, AF.Relu, bias=bias)
                else:
                    nc.vector.tensor_scalar(dst, src_ps, bias, 0.0,
                                            ALU.add, ALU.max)

            def copy_out(dst, src_ps):
                """dst = src_ps (psum evacuate), alternating ACT/DVE."""
                if ew_next():
                    nc.scalar.activation(dst, src_ps, AF.Identity)
                else:
                    nc.vector.tensor_copy(dst, src_ps)

            # =============================================================
            # Phase helpers (generators yield at interleave boundaries)
            # =============================================================
            def kv_chain(l, h, pre):
                """One (layer, head, k-or-v) MLP chain over all V rows."""
                cn = f"{pre}{l}{h}"
                w12 = wpool.tile([128, 4, M], FP8, tag="w12",
                                 name=f"w12{cn}")
                w1c4 = wpool.tile([128, M], BF16, tag="w1c4", name=f"w1c4{cn}")
                w3p = wpool.tile([128, 2 * AD], BF16, tag="w3p",
                                 name=f"w3p{cn}")
                bdt = wpool.tile([128, 4], mybir.dt.float32, tag="bdt",
                                 name=f"bdt{cn}")
                # 2 sync-issued + 2 ACT-issued DMAs (DMA descriptor issue is
                # ~600ns of serial sequencer time -- 8 per chain paced the
                # whole chain phase)
                nc.sync.dma_start(w12[:], kvw[pre + "12"][l, h])
                nc.sync.dma_start(
                    w1c4.rearrange("(r c) m -> r c m", c=32)[:, 0:1, :],
                    kvw[pre + "1c"][l, h])
                nc.scalar.dma_start(w3p[:], kvw[pre + "3"][l, h])
                nc.scalar.dma_start(bdt[:], kvw[pre + "b"][l, h])
                w1 = w12[:, 0:2, :]
                w2 = w12[:, 2:4, :]
                w3a = w3p[:, 0:AD]
                w3b = w3p[:, AD:2 * AD]
                b1 = [bdt[:, fc:fc + 1] for fc in range(2)]
                b2 = [bdt[:, 2 + fc:3 + fc] for fc in range(2)]

                h18 = kpool.tile([128, 2, V], FP8, tag="h18",
                                 name=f"h18{cn}", bufs=3, uniquify=True)
                h2t = {}
                for ntp in range(2):
                    hsl = slice(1024 * ntp, 1024 * (ntp + 1))
                    # one psum alloc per fc half-wave (deeper ring pipelining)
                    for fc in range(2):
                        ps1 = big_tile(f"ps1{cn}{fc}{ntp}")
                        cs = slice(128 * fc, 128 * (fc + 1))
                        # rank-1 true_u term: K=1 bf16, row-packed at
                        # rows 64*fc + 32*j -> concurrent PE sub-arrays
                        for j in range(2):
                            sl = slice(1024 * ntp + 512 * j,
                                       1024 * ntp + 512 * (j + 1))
                            col = slice(512 * j, 512 * (j + 1))
                            row = 64 * fc + 32 * j
                            nc.tensor.matmul(
                                ps1[:, col],
                                w1c4[row:row + 1, 128 * fc:128 * (fc + 1)],
                                u4[row:row + 1, sl],
                                start=True, stop=False,
                                tile_position=(row, 0))
                        for j in range(2):
                            sl = slice(1024 * ntp + 512 * j,
                                       1024 * ntp + 512 * (j + 1))
                            col = slice(512 * j, 512 * (j + 1))
                            nc.tensor.matmul(
                                ps1[:, col], w1[:, :, cs],
                                xt[:, :, sl],
                                start=False, stop=True, perf_mode=DR)
                        relu_out(h18[:, fc, hsl], ps1[:], b1[fc])
                    yield
                for ntp in range(2):
                    for fc in range(2):
                        ps2 = big_tile(f"ps2{cn}{fc}{ntp}")
                        cs = slice(128 * fc, 128 * (fc + 1))
                        for j in range(2):
                            col = slice(512 * j, 512 * (j + 1))
                            sl = slice(1024 * ntp + 512 * j,
                                       1024 * ntp + 512 * (j + 1))
                            nc.tensor.matmul(ps2[:, col], w2[:, :, cs],
                                             h18[:, :, sl],
                                             start=True, stop=True,
                                             perf_mode=DR)
                        t = kpool.tile([128, 1024], BF16, tag="h2",
                                       name=f"h2{cn}{fc}{ntp}", bufs=6,
                                       uniquify=True)
                        relu_out(t[:], ps2[:], b2[fc])
                        h2t[(fc, ntp)] = t
                    yield

                if pre == "k":
                    q, hp = h // 4, h % 4
                    # 4-way column-packed: col tile s covers keys quarter s
                    psk = big_tile(f"psk{cn}")
                    # stationary-major order: 4 col-packed w3a matmuls, then
                    # 4 col-packed w3b (shared stationary, concurrent cols)
                    for s in range(4):
                        ntp, j = s // 2, s % 2
                        col = slice(512 * j, 512 * (j + 1))
                        nc.tensor.matmul(psk[32 * s:32 * (s + 1), 0:512],
                                         w3a, h2t[(0, ntp)][:, col],
                                         start=True, stop=False,
                                         tile_position=(0, 32 * s))
                    for s in range(4):
                        ntp, j = s // 2, s % 2
                        col = slice(512 * j, 512 * (j + 1))
                        nc.tensor.matmul(psk[32 * s:32 * (s + 1), 0:512],
                                         w3b, h2t[(1, ntp)][:, col],
                                         start=False, stop=True,
                                         tile_position=(0, 32 * s))
                    # kb3 cancels in softmax (constant per (p,h)) -> no bias
                    for s in range(4):
                        dst = kt[l][q][32 * hp:32 * (hp + 1),
                                       512 * s:512 * (s + 1)]
                        copy_out(dst, psk[32 * s:32 * (s + 1), 0:512])
                else:
                    psv = big_tile(f"psv{cn}")
                    for svg in range(16):
                        ntp, w = svg // 8, svg % 8
                        j, c = w // 4, w % 4
                        vsl = slice(512 * j + 128 * c,
                                    512 * j + 128 * (c + 1))
                        osl = slice(32 * svg, 32 * (svg + 1))
                        nc.tensor.matmul(
                            psv[:, osl], h2t[(0, ntp)][:, vsl], w3a,
                            start=(svg == 0), stop=False)
                        nc.tensor.matmul(
                            psv[:, osl], h2t[(1, ntp)][:, vsl], w3b,
                            start=False, stop=(svg == 15))
                    vt = vals[l][h]
                    nc.vector.tensor_copy(
                        vt[:, :, 0:AD],
                        psv[:, 0:512].rearrange("p (s d) -> p s d", d=AD))
                    nc.vector.memset(vt[:, :, AD:AD + 1], 1.0)
                yield

            def split_bf(src_tiles, tagp, need_lo=True):
                """f32 [128,P] tiles -> (hi bf16, lo bf16) tiles."""
                his, los = [], []
                for q, s in enumerate(src_tiles):
                    hi = kpool.tile([128, P], BF16, tag=f"{tagp}h{q}",
                                    name=f"{tagp}h{q}", bufs=2, uniquify=True)
                    nc.vector.tensor_copy(hi[:], s[:])
                    his.append(hi)
                    if need_lo:
                        lo = kpool.tile([128, P], BF16, tag=f"{tagp}l{q}",
                                        name=f"{tagp}l{q}", bufs=2,
                                        uniquify=True)
                        nc.vector.tensor_tensor(lo[:], s[:], hi[:],
                                                ALU.subtract)
                        los.append(lo)
                return his, los

            def layer_norm(xq, pfx, l, nm, box):
                """T-layout LN over 256 features (generator; appends two
                f32 tiles to box)."""
                xh, _ = split_bf(xq, "lnx", need_lo=False)
                psum2 = big_tile(f"lnsums{nm}")
                pst = psum2[0:1, 0:512]
                psq = psum2[0:1, 512:1024]
                nc.tensor.matmul(pst, ones_c128b[:], xh[0][:],
                                 start=True, stop=False)
                nc.tensor.matmul(pst, ones_c128b[:], xh[1][:],
                                 start=False, stop=True)
                sq = [kpool.tile([128, P], BF16, tag=f"lnsq{q}",
                                 name=f"lnsq{nm}{q}", bufs=1)
                      for q in range(2)]
                for q in range(2):
                    nc.vector.tensor_tensor(sq[q][:], xh[q][:], xh[q][:],
                                            ALU.mult)
                nc.tensor.matmul(psq, ones_c128b[:], sq[0][:],
                                 start=True, stop=False)
                nc.tensor.matmul(psq, ones_c128b[:], sq[1][:],
                                 start=False, stop=True)
                yield
                mu = kpool.tile([1, P], mybir.dt.float32, tag="lnmu", bufs=1,
                                name=f"lnmu{nm}")
                nc.scalar.mul(mu[:], pst, 1.0 / D)
                m2 = kpool.tile([1, P], mybir.dt.float32, tag="lnm2", bufs=1,
                                name=f"lnm2{nm}")
                nc.vector.tensor_tensor(m2[:], mu[:], mu[:], ALU.mult)
                var = kpool.tile([1, P], mybir.dt.float32, tag="lnvar",
                                 bufs=1, name=f"lnvar{nm}")
                nc.vector.scalar_tensor_tensor(
                    var[:], psq, 1.0 / D, m2[:], ALU.mult, ALU.subtract)
                # rstd = exp(-0.5*ln(var+eps)) -- Ln/Exp share one ACT table
                # set (no Sqrt-set reload, no slow DVE reciprocal)
                lnv = kpool.tile([1, P], mybir.dt.float32, tag="lnsd", bufs=1,
                                 name=f"lnsd{nm}")
                nc.scalar.activation(lnv[:], var[:], AF.Ln, bias=eps_t[:])
                rstd = kpool.tile([1, P], mybir.dt.float32, tag="lnrs",
                                  bufs=1, name=f"lnrs{nm}")
                nc.scalar.activation(rstd[:], lnv[:], AF.Exp, scale=-0.5)
                nmu = kpool.tile([1, P], mybir.dt.float32, tag="lnnm", bufs=1,
                                 name=f"lnnm{nm}")
                nc.vector.scalar_tensor_tensor(
                    nmu[:], mu[:], -1.0, rstd[:], ALU.mult, ALU.mult)
                # bf16 copies: the broadcast matmuls run at full rate (f32
                # matmuls are half-rate with slow LDWs)
                rsb = kpool.tile([1, 2, P], BF16, tag="lnrsb", bufs=1,
                                 name=f"lnrsb{nm}")
                nc.vector.tensor_copy(rsb[:, 0, :], rstd[:])
                nc.vector.tensor_copy(rsb[:, 1, :], nmu[:])
                yield
                # broadcast A=rstd, B=-mu*rstd to 128 partitions
                psab = big_tile(f"lnAB{nm}")
                nc.tensor.matmul(psab[:, 0:512], ones_rb16[:],
                                 rsb[:, 0, :], start=True, stop=True)
                nc.tensor.matmul(psab[:, 512:1024], ones_rb16[:],
                                 rsb[:, 1, :], start=True, stop=True)
                for q in range(2):
                    g = lnw[(pfx + "g", l, q)]
                    bb = lnw[(pfx + "b", l, q)]
                    t1 = kpool.tile([128, P], mybir.dt.float32, tag=f"lnt{q}",
                                    name=f"lnt{nm}{q}", bufs=1)
                    nc.vector.tensor_tensor(t1[:], xq[q][:], psab[:, 0:512],
                                            ALU.mult)
                    nc.vector.tensor_tensor(t1[:], t1[:], psab[:, 512:1024],
                                            ALU.add)
                    o = kpool.tile([128, P], mybir.dt.float32, tag=f"attv{q}",
                                   name=f"ln_out{nm}{q}", bufs=2)
                    nc.vector.tensor_scalar(o[:], t1[:], g[:], bb[:],
                                            ALU.mult, ALU.add)
                    box.append(o)
                yield

            def attention(l, attv, out_box, av_lag=0):
                """Count-matrix softmax attention (generator; yields per vc).
                4-way row-packed QK per quad; appends LN output to out_box.
                av_lag delays AV emission so early steps need no vals yet."""
                qt_h, _ = split_bf(attv, "qt", need_lo=False)
                numer = [kpool.tile([128, P], mybir.dt.float32, tag=f"num{q}",
                                    name=f"numer{l}{q}", bufs=1)
                         for q in range(2)]
                xres = []

                def ep_tail(q, dn4):
                    """recip (exp(-ln)) + broadcast + residual for quad q --
                    deferred so PE work can be emitted in between."""
                    lnd = kpool.tile([4, P], mybir.dt.float32, tag="lnd",
                                     name=f"lnd{l}{q}", bufs=2)
                    nc.scalar.activation(lnd[:], dn4[:], AF.Ln)
                    rd4 = kpool.tile([4, P], BF16, tag="rd4",
                                     name=f"rd4{l}{q}", bufs=2)
                    nc.scalar.activation(rd4[:], lnd[:], AF.Exp, scale=-1.0)
                    psrb = big_tile(f"psrb{l}{q}")
                    nc.tensor.matmul(psrb[:, 0:512], oh4[:], rd4[:],
                                     start=True, stop=True)
                    t1 = kpool.tile([128, P], mybir.dt.float32,
                                    tag=f"xres{q}", name=f"xres{l}{q}",
                                    bufs=1)
                    nc.vector.tensor_tensor(t1[:], numer[q][:],
                                            psrb[:, 0:512], ALU.mult)
                    # + attv (residual) + vb3 (value-bias; softmax wts sum=1)
                    nc.vector.scalar_tensor_tensor(
                        t1[:], t1[:], vb3t[(l, q)][:], attv[q][:],
                        ALU.add, ALU.add)
                    xres.append(t1)

                def emit_av(q, psA, vc, es):
                    for g in range(2):
                        h0 = 4 * q + 2 * g
                        nc.tensor.matmul(
                            psA[g][0:AD + 1, :],
                            vals[l][h0][:, vc, :],
                            es[g][:, 0:512],
                            start=(vc == 0), stop=(vc == 15))
                        nc.tensor.matmul(
                            psA[g][64:64 + AD + 1, :],
                            vals[l][h0 + 1][:, vc, :],
                            es[g][:, 512:1024],
                            start=(vc == 0), stop=(vc == 15),
                            tile_position=(0, 64))

                dn4s = [None, None]
                for q in range(2):
                    psA = [psA_tile(f"psA{l}{q}{g}") for g in range(2)]
                    avq = []
                    for vc in range(16):
                        # all 4 QKs back-to-back: rows 0/32/64/96 pack
                        # concurrently in the PE sub-arrays
                        psst = [big_tile(f"pss{l}{q}{g}{vc}")
                                for g in range(2)]
                        for g in range(2):
                            for hp in (2 * g, 2 * g + 1):
                                bb = 32 * hp
                                tp = (bb, 0) if bb >= 64 else None
                                col = slice(512 * (hp % 2),
                                            512 * (hp % 2 + 1))
                                nc.tensor.matmul(
                                    psst[g][:, col],
                                    kt[l][q][bb:bb + 32,
                                             128 * vc:128 * (vc + 1)],
                                    qt_h[q][bb:bb + 32, :],
                                    start=True, stop=True, tile_position=tp)
                        es = []
                        for g in range(2):
                            e = kpool.tile([128, 1024], BF16, tag="ebuf",
                                           name=f"e{l}{q}{g}{vc}", bufs=3,
                                           uniquify=True)
                            nc.scalar.activation(e[:], psst[g][:], AF.Exp,
                                                 scale=SCALE)
                            ce = kpool.tile([128, 1024], BF16, tag="cebuf",
                                            name=f"ce{l}{q}{g}{vc}", bufs=12,
                                            uniquify=True)
                            nc.vector.tensor_tensor(
                                ce[:], e[:],
                                ct[:, vc:vc + 1, :].broadcast_to(
                                    [128, 2, P]),
                                ALU.mult)
                            es.append(ce)
                        avq.append((vc, es))
                        if len(avq) > av_lag:
                            emit_av(q, psA, *avq.pop(0))
                        yield
                        if q == 1 and vc == 2:
                            ep_tail(0, dn4s[0])
                    for vc_, es_ in avq:
                        emit_av(q, psA, vc_, es_)
                    # psA evacuation inline (frees the AV psum ring for the
                    # next quad); the recip/broadcast tail is deferred
                    dn4 = kpool.tile([4, P], mybir.dt.float32, tag="dn4",
                                     name=f"dn4{l}{q}", bufs=2)
                    for g in range(2):
                        b0, b1r = 32 * (2 * g), 32 * (2 * g + 1)
                        copy_out(numer[q][b0:b0 + 32, :], psA[g][0:32, :])
                        copy_out(numer[q][b1r:b1r + 32, :], psA[g][64:96, :])
                        for hl, prow in ((2 * g, 32), (2 * g + 1, 96)):
                            dtmp = kpool.tile([1, P], mybir.dt.float32,
                                              tag="dntmp", bufs=2,
                                              name=f"dtmp{l}{q}{hl}",
                                              uniquify=True)
                            copy_out(dtmp[:], psA[g][prow:prow + 1, :])
                            nc.sync.dma_start(dn4[hl:hl + 1, :], dtmp[:])
                    dn4s[q] = dn4
                yield
                ep_tail(1, dn4s[1])
                yield
                yield from layer_norm(xres, "ln1", l, f"ln1_{l}", out_box)

            def ff_block(l, attv, out_box):
                """Feed-forward block (generator; yields between stages)."""
                av16 = []
                for kc in range(2):
                    t = kpool.tile([128, P], FP16, tag=f"ff16{kc}",
                                   name=f"ffav{l}{kc}", bufs=2)
                    nc.vector.tensor_copy(t[:], attv[kc][:])
                    av16.append(t)
                yield
                hh_t = []
                for fc in range(2):
                    psf = big_tile(f"psff1{l}{fc}")
                    for kc in range(2):
                        nc.tensor.matmul(
                            psf[:, 0:512],
                            fw1[l][kc][:, 128 * fc:128 * (fc + 1)],
                            av16[kc][:], start=(kc == 0), stop=(kc == 1))
                    th = kpool.tile([128, P], FP16, tag=f"ffhh{fc}",
                                    name=f"ffhh{l}{fc}", bufs=2)
                    relu_out(th[:], psf[:, 0:512], ffb[("b1", l, fc)][:])
                    hh_t.append(th)
                    yield
                xres2 = []
                for fc in range(2):
                    psf2 = big_tile(f"psff2{l}{fc}")
                    for kc in range(2):
                        nc.tensor.matmul(
                            psf2[:, 0:512],
                            fw2[l][kc][:, 128 * fc:128 * (fc + 1)],
                            hh_t[kc][:], start=(kc == 0), stop=(kc == 1))
                    t2 = kpool.tile([128, P], mybir.dt.float32,
                                    tag=f"xres{fc}", name=f"xr2{l}{fc}",
                                    bufs=1)
                    nc.vector.scalar_tensor_tensor(
                        t2[:], psf2[:, 0:512], ffb[("b2", l, fc)][:],
                        attv[fc][:], ALU.add, ALU.add)
                    xres2.append(t2)
                    yield
                yield from layer_norm(xres2, "ln2", l, f"ln2_{l}", out_box)

            # =============================================================
            # Emit program
            # =============================================================
            # ds projection: attv0 = cur @ ds_W + ds_b   (T-layout out);
            # deferred into the v0 phase to keep startup DMAs off the
            # first chains' critical path
            dsw16 = [cpool.tile([128, D], FP16, name=f"dsw16{kc}")
                     for kc in range(2)]
            dsb = [cpool.tile([128, 1], F32, name=f"dsb{q}")
                   for q in range(2)]
            attv = []

            def emit_ds():
                for kc in range(2):
                    nc.sync.dma_start(dsw16[kc][:],
                                      dsw16d[128 * kc:128 * (kc + 1), :])
                    nc.sync.dma_start(cur16[kc][:],
                                      cur16d[128 * kc:128 * (kc + 1), :])
                for q in range(2):
                    nc.sync.dma_start(dsb[q][:], dsbd[q, :, :])

            def emit_ds2():
                for q in range(2):
                    psd = big_tile(f"psds{q}")
                    for kc in range(2):
                        nc.tensor.matmul(
                            psd[:, 0:512],
                            dsw16[kc][:, 128 * q:128 * (q + 1)],
                            cur16[kc][:], start=(kc == 0), stop=(kc == 1))
                    o = kpool.tile([128, P], mybir.dt.float32,
                                   tag=f"attv{q}", name=f"attv0{q}", bufs=2)
                    nc.vector.tensor_scalar(o[:], psd[:, 0:512], dsb[q][:],
                                            None, ALU.add)
                    attv.append(o)

            # --- pipelined emission ------------------------------------
            def drain(g):
                for _ in g:
                    pass

            def weave_gen(gens, after=()):
                """Cross-chain pipeline (generator): chain c's mm3 tail is
                emitted right after chain c+1's first mm1 wave.  `after` is
                a sequence of (chain_idx, fn) emission hooks."""
                hooks = dict(after)
                tail = None
                for idx, g in enumerate(gens):
                    next(g)
                    yield
                    if tail is not None:
                        for _ in tail:
                            yield
                    for _ in range(3):
                        next(g)
                        yield
                    tail = g
                    if idx in hooks:
                        hooks[idx]()
                if tail is not None:
                    for _ in tail:
                        yield

            def par2(agen, asteps, cgen, ratio):
                """Advance agen by asteps yields, interleaving ~ratio chain
                steps per attention step."""
                for _ in range(asteps):
                    try:
                        next(agen)
                    except StopIteration:
                        break
                    if cgen is not None:
                        for _ in range(ratio):
                            try:
                                next(cgen)
                            except StopIteration:
                                cgen = None
                                break

            k0 = [kv_chain(0, hh, "k") for hh in range(H)]
            v0 = [kv_chain(0, hh, "v") for hh in range(H)]
            k1 = [kv_chain(1, hh, "k") for hh in range(H)]
            v1 = [kv_chain(1, hh, "v") for hh in range(H)]

            drain(weave_gen(k0[0:4], after=(
                (1, lambda: (hoists[0](), emit_ds())), (2, hoists[1]),
                (3, hoists[2]))))
            # attn0-q0's QK/exp/ce need only kt0[q0] + attv; with av_lag=6
            # its first steps overlap the v0 chains that produce vals0.
            c_head = weave_gen(v0[0:4], after=(
                (0, hoists[3]), (1, lambda: (hoists[4](), emit_ds2()))))
            for _ in range(12):
                next(c_head)
            box0 = []
            a0 = attention(0, attv, box0, av_lag=5)
            par2(a0, 6, c_head, ratio=2)
            drain(c_head)
            # q1 needs k0[4:]+v0[4:] -> fully emitted within q0's steps.
            c1a = weave_gen(k0[4:8] + v0[4:8],
                            after=((1, hoists[5]), (3, hoists[6])))
            par2(a0, 10, c1a, ratio=4)
            drain(c1a)
            # q1 + tail interleave with k1 (attn1-q0 needs all of kt1
            # emitted before attention(1) starts).
            c1b = weave_gen(k1)
            par2(a0, 21, c1b, ratio=2)
            drain(a0)
            drain(c1b)
            attv = box0
            # v1 chains fill the PE-idle ff0/attn1 windows: v1[0:4] must be
            # fully emitted before attn1's first AV (quad0), v1[4:8] before
            # its quad-1 AVs.
            c1ca = weave_gen(v1[0:4])
            box_f0 = []
            f0 = ff_block(0, attv, box_f0)
            par2(f0, 8, c1ca, ratio=3)
            drain(f0)
            drain(c1ca)
            attv = box_f0

            box1 = []
            a1 = attention(1, attv, box1, av_lag=4)
            c1cb = weave_gen(v1[4:8])
            par2(a1, 37, c1cb, ratio=2)
            drain(c1cb)
            drain(a1)
            attv = box1
            box_f1 = []
            drain(ff_block(1, attv, box_f1))
            attv = box_f1

            # ---- decoder ----------------------------------------------
            de16 = []
            for kc in range(2):
                t = kpool.tile([128, P], FP16, tag=f"de16{kc}",
                               name=f"de16{kc}", bufs=1)
                nc.vector.tensor_copy(t[:], attv[kc][:])
                de16.append(t)
            d1 = []
            for fc in range(2):
                psd1 = big_tile(f"psde1{fc}")
                for kc in range(2):
                    nc.tensor.matmul(
                        psd1[:, 0:512],
                        dw1[kc][:, 128 * fc:128 * (fc + 1)],
                        de16[kc][:], start=(kc == 0), stop=(kc == 1))
                th = kpool.tile([128, P], FP16, tag=f"d1h{fc}",
                                name=f"d1h{fc}", bufs=2)
                relu_out(th[:], psd1[:, 0:512], deb1[fc][:])
                d1.append(th)
            d2 = []
            for fc in range(2):
                psd2 = big_tile(f"psde2{fc}")
                for kc in range(2):
                    nc.tensor.matmul(
                        psd2[:, 0:512],
                        dw2[kc][:, 128 * fc:128 * (fc + 1)],
                        d1[kc][:], start=(kc == 0), stop=(kc == 1))
                th = kpool.tile([128, P], FP16, tag=f"d2h{fc}",
                                name=f"d2h{fc}", bufs=2)
                relu_out(th[:], psd2[:, 0:512], deb2[fc][:])
                d2.append(th)

            # logits row-major [p, R] per 128-p chunk + loss.  Targets are
            # host-precomputed one-hots; logits are O(10) so exp needs no
            # max-shift in f32.
            oht = kpool.tile([128, 4, R], mybir.dt.float32, tag="oht",
                             name="oht", bufs=1)
            nc.sync.dma_start(oht[:], ohtd[:])
            t4 = kpool.tile([128, 4], mybir.dt.float32, tag="t4",
                            name="t4", bufs=1)

            def logits_tail(pc, psl):
                escr = kpool.tile([128, R], mybir.dt.float32, tag="escr",
                                  bufs=2, name=f"escr{pc}")
                se = kpool.tile([128, 1], mybir.dt.float32, tag="se",
                                name=f"se{pc}", bufs=2)
                nc.scalar.activation(escr[:], psl[:, 0:R], AF.Exp,
                                     accum_out=se[:])
                ls = kpool.tile([128, 1], mybir.dt.float32, tag="ls",
                                name=f"ls{pc}", bufs=2)
                nc.scalar.activation(ls[:], se[:], AF.Ln)
                scr2 = kpool.tile([128, R], mybir.dt.float32, tag="scr2",
                                  bufs=2, name=f"scr2{pc}")
                pk = kpool.tile([128, 1], mybir.dt.float32, tag="pk",
                                name=f"pk{pc}", bufs=2)
                nc.vector.scalar_tensor_tensor(
                    scr2[:], psl[:, 0:R], 1.0, oht[:, pc, :],
                    ALU.mult, ALU.mult, accum_out=pk[:])
                nc.vector.tensor_tensor(t4[:, pc:pc + 1], pk[:], ls[:],
                                        ALU.subtract)

            prev_log = None
            for pc in range(4):
                psl = big_tile(f"pslog{pc}")
                psl_sl = slice(128 * pc, 128 * (pc + 1))
                # bias row (f32 rank-1), then the two K-chunks
                nc.tensor.matmul(psl[:, 0:R], ones_rbf[:], db3f[:],
                                 start=True, stop=False)
                for kc in range(2):
                    nc.tensor.matmul(psl[:, 0:R], d2[kc][:, psl_sl],
                                     dw3[kc][:],
                                     start=False, stop=(kc == 1))
                if prev_log is not None:
                    logits_tail(*prev_log)
                prev_log = (pc, psl)
            logits_tail(*prev_log)
            pspr = big_tile("pspr")
            nc.tensor.matmul(pspr[0:1, 0:4], ones_c128f[:], t4[:],
                             start=True, stop=True)
            pr4 = kpool.tile([1, 4], mybir.dt.float32, tag="pr4",
                             name="pr4", bufs=1)
            nc.vector.tensor_copy(pr4[:], pspr[0:1, 0:4])
            s1 = kpool.tile([1, 1], mybir.dt.float32, tag="s1",
                            name="s1", bufs=1)
            nc.vector.tensor_reduce(s1[:], pr4[:], mybir.AxisListType.X,
                                    ALU.add)
            outt = kpool.tile([1, 1], mybir.dt.float32, tag="outt",
                              name="outt", bufs=1)
            nc.scalar.activation(outt[:], s1[:], AF.Identity,
                                 bias=nlogr_t[:], scale=-1.0)
            nc.sync.dma_start(out_d[:], outt[:])

    return nc


F8 = getattr(ml_dtypes, "float8_e4m3", ml_dtypes.float8_e4m3fn)


def _kpack(w, dt=None):
    """[256, F] -> K-halves packed [128, 2, F]."""
    w = np.asarray(w, np.float32)
    return np.ascontiguousarray(
        w.reshape(2, 128, w.shape[-1]).transpose(1, 0, 2)).astype(dt or F8)


def _maybe_enable_trace():
    """Optional NTFF profiling under axon (KERNEL_TRACE=1); best-effort."""
    try:
        import sys
        import types

        import antenv

        if "antenv.axon_hooks" not in sys.modules:
            mod = types.ModuleType("antenv.axon_hooks")
            mod._hook = None
            mod.set_axon_ntff_profile_hook = lambda h: setattr(mod, "_hook", h)
            mod.get_axon_ntff_profile_hook = lambda: mod._hook
            sys.modules["antenv.axon_hooks"] = mod
            antenv.axon_hooks = mod
            from trn_agent_boot.trn_boot import _ntff_profile_via_ctypes

            mod._hook = _ntff_profile_via_ctypes("/opt/axon/libaxon_pjrt.so")
        import concourse.bass_utils as _bu

        _bu.upload_artifacts = lambda tmpdir: f"file://{tmpdir}"
        return True
    except Exception:
        return False


LAST_RESULT = {}


def _enable_ldw_opt():
    """Turn on walrus LDWEIGHTS dedup (consecutive matmuls sharing a
    stationary tile skip the reload) for our own compile invocation."""
    import concourse.bass_utils as _bu

    if getattr(_bu, "_ldw_opt_patched", False):
        return
    orig = _bu.run_command

    # ldw-opt errors out on tile_position LDWs ("not compatible with LDW
    # optimization") -- left disabled.
    _ = orig
    _bu._ldw_opt_patched = True


def kernel(**inputs):
    from concourse.bass_utils import run_bass_kernel_spmd
    _enable_ldw_opt()

    if "nc" not in _BUILT:
        _BUILT["nc"] = _build()
    nc = _BUILT["nc"]

    f32 = lambda a: np.ascontiguousarray(np.asarray(a, np.float32))
    bf = lambda a: np.ascontiguousarray(np.asarray(a, np.float32)).astype(BF)
    f16 = lambda a: np.ascontiguousarray(
        np.asarray(a, np.float32)).astype(np.float16)

    enc = f32(inputs["encoded"])                      # [B,V,I]
    tu = f32(inputs["true_u"])                        # [B,V,1]
    mask = f32(inputs["attn_mask"])                   # [P,N]
    pp_ = np.asarray(inputs["pred_points"]).astype(np.int64)
    ni = np.asarray(inputs["neighbor_index"]).astype(np.int64)

    # count matrix C[p, v]
    C = np.zeros((P, V), np.float32)
    np.add.at(C, (np.repeat(np.arange(P), N), ni.ravel()),
              np.exp(-SCALE * mask).ravel().astype(np.float32))
    ctm = np.ascontiguousarray(C.T).astype(BF)        # [V, P]

    shared = {"ctm": ctm}
    for pre in ("k", "v"):
        w1 = f32(inputs[pre + "W1"])                  # [L,H,257,256]
        w2 = f32(inputs[pre + "W2"])
        shared[pre + "w12"] = np.stack(
            [np.stack([np.concatenate(
                [_kpack(w1[l, h, 0:256]), _kpack(w2[l, h])], axis=1)
                for h in range(H)]) for l in range(L)])
        # u rank-1 row replicated 4x for 4-way row packing
        shared[pre + "1c"] = np.ascontiguousarray(np.broadcast_to(
            w1[:, :, 256:257, :], (L, H, 4, M))).astype(BF)
        w3 = f32(inputs[pre + "W3"])                  # [L,H,256,32]
        shared[pre + "w3"] = np.concatenate(
            [w3[:, :, 0:128, :], w3[:, :, 128:256, :]], axis=3).astype(BF)
        b1 = f32(inputs[pre + "b1"]).reshape(L, H, 2, 128)
        b2 = f32(inputs[pre + "b2"]).reshape(L, H, 2, 128)
        shared[pre + "bb"] = np.ascontiguousarray(
            np.stack([b1[:, :, 0], b1[:, :, 1], b2[:, :, 0], b2[:, :, 1]],
                     axis=3))
    # rename to match din names
    shared["kw1c"] = shared.pop("k1c")
    shared["vw1c"] = shared.pop("v1c")
    # vb3 folded into residual (softmax weights sum to 1); kb3 cancels.
    shared["vb3qd"] = np.ascontiguousarray(
        f32(inputs["vb3"]).reshape(L, 2, 128)[:, :, :, None])

    shared["dsw16d"] = f16(inputs["ds_W"])
    shared["dsbd"] = f32(inputs["ds_b"]).reshape(2, 128, 1)
    shared["ffw1d"] = f16(inputs["ff_W1"])
    shared["ffw2d"] = f16(inputs["ff_W2"])
    shared["ffb1d"] = f32(inputs["ff_b1"]).reshape(L, 2, 128, 1)
    shared["ffb2d"] = f32(inputs["ff_b2"]).reshape(L, 2, 128, 1)
    shared["ln1gd"] = f32(inputs["ln1_g"]).reshape(L, 2, 128, 1)
    shared["ln1bd"] = f32(inputs["ln1_b"]).reshape(L, 2, 128, 1)
    shared["ln2gd"] = f32(inputs["ln2_g"]).reshape(L, 2, 128, 1)
    shared["ln2bd"] = f32(inputs["ln2_b"]).reshape(L, 2, 128, 1)
    shared["dew1d"] = f16(inputs["de_W1"])
    shared["dew2d"] = f16(inputs["de_W2"])
    shared["dew3d"] = f16(inputs["de_W3"])
    shared["deb1d"] = f32(inputs["de_b1"]).reshape(2, 128, 1)
    shared["deb2d"] = f32(inputs["de_b2"]).reshape(2, 128, 1)
    shared["deb3d"] = f32(inputs["de_b3"]).reshape(1, R)

    oh4f = np.zeros((4, 128), np.float32)
    for i in range(4):
        oh4f[i, 32 * i:32 * (i + 1)] = 1.0
    shared["oh8d"] = oh4f.astype(BF)

    in_maps = []
    for b in range(B):
        merged = np.concatenate([enc[b], tu[b]], axis=1)  # [V, 257]
        mt = np.ascontiguousarray(merged.T)               # [257, V]
        cur = enc[b][pp_, :]                              # [P, I]
        curt = np.ascontiguousarray(cur.T)                # [I, P]
        m = dict(shared)
        m["xtd"] = _kpack(mt[0:256])
        m["xt2"] = np.ascontiguousarray(
            np.broadcast_to(mt[256:257], (4, V))).astype(BF)
        m["cur16d"] = curt.astype(np.float16)
        m["updc"] = tu[b][pp_, :]                          # [P,1] f32
        tgt = np.clip(np.floor(tu[b][pp_, 0] * R).astype(np.int64), 0, R - 1)
        oht = np.zeros((P, R), np.float32)
        oht[np.arange(P), tgt] = 1.0
        # oht[p_local, pc, r]
        m["ohtd"] = np.ascontiguousarray(
            oht.reshape(4, 128, R).transpose(1, 0, 2))
        in_maps.append(m)

    trace = os.environ.get("KERNEL_TRACE") == "1" and _maybe_enable_trace()
    res = run_bass_kernel_spmd(
        nc, in_maps, core_ids=list(range(B)), trace=trace,
        trace_cores=list(range(B)) if trace else None)
    LAST_RESULT["res"] = res
    if trace and res.exec_time_ns is not None:
        print(f"HW exec time: {res.exec_time_ns} ns "
              f"(mean {res.mean_exec_time_ns} ns, "
              f"slowest core {res.max_exec_time_core_id})")
    out = np.array([res.results[b]["out"][0, 0] for b in range(B)], np.float32)
    return out


# revision 68
# speedup vs baseline: 1.0200x; 1.0200x over previous
"""Trainium2 Bass kernel for nn_CopulaDecoder.  (HW: ~508us, rel err 3.7e-3)

Data-parallel over batch: core b computes batch element b end-to-end.
All activations live transposed (features on partitions, tokens on free dim).
The neighbor-gather softmax is reformulated as a dense count-matrix softmax:
  softmax over the 64 gathered scores == (C * exp(scale*S)) normalized, where
  C[p,v] = sum_n 1[neighbor_index[p,n]==v] * exp(-scale*attn_mask[p,n]).
Scores are small (|scale*S| < ~4 for this model family), so no max-shift.

Bias algebra: the key-MLP output bias kb3 shifts every score for a given
(p, head) by the same constant q.kb3 -> cancels in softmax -> dropped.
The value-MLP output bias vb3 shifts the attention output by vb3 (softmax
weights sum to 1) -> folded into the residual add before LN1.  Target
one-hots for the final NLL are host-precomputed; logits are O(10) so the
log-softmax needs no max-shift.  All reciprocals (LN rstd, softmax denom)
are exp(-ln x) / exp(-0.5 ln(x+eps)) on ACT: Ln+Exp live in one activation
table set (natural_log_exp), so the kernel never reloads ACT tables and
avoids the 8-cycle/elem DVE divide.

Precision: fp8-e4m3 DoubleRow (K=256/pass) for the two big KV-MLP matmuls,
bf16 for mm3 + attention, fp16 single for the small matmuls (ds/ff/decoder)
with f32 per-partition biases applied in the epilogue ops (no bias_mm
rank-1s, no hi/lo splits), fp32 accumulate, fp32 elementwise.

Scheduling: one software pipeline across the whole net.  Emission order ==
engine queue order (in-order engines), so overlap is achieved by
interleaving emission subject to producer-before-consumer order:
  k0[0:4] -> [attn0-q0 (av_lag) || v0] -> [attn0 || k0/v0 tails, k1] ->
  [ff0 || v1[0:4]] -> [attn1 || v1[4:8]] -> ff1 -> decoder (pipelined
  logits).  DMA hoists (count matrix in 16 pieces, weights) are spread
  across the head chains so no transfer ever blocks chain weights.  DMA
  descriptor issue costs ~600ns of serial sequencer time, so chain weights
  arrive in 2 sync-issued DMAs (w1+w2 packed into one fp8 tensor; the 4
  rank-1 rows as one partition-strided transfer) + 2 ACT-issued ones.
attention defers AV emission by av_lag steps (deep ce ring) so its
QK/exp/ce can overlap chains that produce the values; the per-quad
denominator pipeline (psA evacuate -> recip -> broadcast -> residual) is
deferred into the next quad's steps to avoid head-of-line PE stalls.
The true_u rank-1 (K=1) matmuls are row-packed pairs (rows 64fc+32j) that
run concurrently in the PE sub-arrays; QK is 4-way row-packed with all
four matmuls emitted back-to-back; AV/mm3-k use column packing.
PSUM: one [128,1024] ring (bufs=3) shared by chains/scores/misc + a
[128,512] AV ring (bufs=2).
"""
import os

import numpy as np
import ml_dtypes

B, S, T = 8, 32, 64
V = S * T
P = 512
N = 2 * S
I = 256
H, AD = 8, 32
D = H * AD
M = 256
L = 2
R = 128
SCALE = float(AD) ** -0.5

BF = ml_dtypes.bfloat16

_BUILT = {}


# ---------------------------------------------------------------------------
# walrus wait-slot workaround (inlined; see dev notes): Tile attaches >1
# semaphore wait to one instruction; many ISA encodings have a single wait
# slot.  Peel excess waits onto injected same-engine InstNoOps.
# ---------------------------------------------------------------------------
def _install_waitfix():
    import bass_rust
    import concourse.mybir as mybir
    import concourse.tile as tile_mod

    if getattr(tile_mod.TileContext, "_waitfix_installed", False):
        return
    limits = {"InstDrain": 1000, "InstEventSemaphore": 1000, "InstCall": 1000,
              "InstISA": 0}
    counter = [0]
    orig_add = tile_mod.TileContext._add_instruction

    def patched_add(self, inst):
        si = inst.sync_info
        if si is not None:
            limit = limits.get(type(inst).__name__, 1)
            waits = list(si.on_wait)
            if len(waits) > limit:
                keep = waits[-limit:] if limit else []
                excess = waits[:-limit] if limit else waits
                while excess:
                    chunk, excess = excess[:1], excess[1:]
                    counter[0] += 1
                    nop = bass_rust.InstNoOp(
                        name=f"waitsplit-{counter[0]}", ins=[], outs=[])
                    nop.engine = inst.engine
                    nop.sync_info = mybir.SyncInfo(on_wait=chunk, on_update=[])
                    orig_add(self, nop)
                inst.sync_info = mybir.SyncInfo(
                    on_wait=keep, on_update=list(si.on_update))
        orig_add(self, inst)

    def patched_drain_and_barrier(self, tick_clock, wait_clock):
        from concourse.tile import ScopedClock

        drain_inst = self.nc.sync.drain()
        wait_clock.add_sem_waits(
            drain_inst.ins, ScopedClock({None: tick_clock.global_clock}))
        si = drain_inst.ins.sync_info
        if si is not None and len(si.on_wait) > 1:
            waits = list(si.on_wait)
            drain_inst.ins.sync_info = mybir.SyncInfo(
                on_wait=waits[:1], on_update=list(si.on_update))
            rest = waits[1:]
            while rest:
                chunk, rest = rest[:1], rest[1:]
                nop = self.nc.sync.nop()
                nop.ins.sync_info = mybir.SyncInfo(on_wait=chunk, on_update=[])
        self.nc.all_engine_barrier()
        assert self.sems is not None
        popped = self.nc._tile_sem_poison_stack.pop()
        assert popped is self._sem_poison
        self.nc.clear_and_free_semaphores(list(self.sems.allocated().values()))
        self.nc.all_engine_barrier()

    try:
        import concourse.tile_utils as tile_utils
        tile_utils.max_sbuf_usage = 204 * 1024
    except Exception:
        pass
    tile_mod.TileContext._add_instruction = patched_add
    tile_mod.TileContext._drain_and_barrier = patched_drain_and_barrier
    tile_mod.TileContext._waitfix_installed = True


def _build():
    """Emit the single-core Bass program (SPMD across 8 cores)."""
    import concourse.bass as bass
    import concourse.mybir as mybir
    import concourse.tile as tile

    _install_waitfix()

    F32 = mybir.dt.float32
    BF16 = mybir.dt.bfloat16
    FP16 = mybir.dt.float16
    FP8 = mybir.dt.float8e4
    DR = mybir.MatmulPerfMode.DoubleRow
    AF = mybir.ActivationFunctionType
    ALU = mybir.AluOpType

    nc = bass.Bass()

    def din(name, shape, dt=BF16):
        return nc.dram_tensor(name, list(shape), dt, kind="ExternalInput")

    # --- DRAM inputs -------------------------------------------------------
    xtd = din("xtd", [128, 2, V], mybir.dt.float8e4)  # merged.T rows 0:256
    xt2 = din("xt2", [4, V])              # row 256 (true_u)            (bf16)
    ctm = din("ctm", [V, P])              # count matrix transposed     (bf16)
    cur16d = din("cur16d", [I, P], FP16)  # cur.T                       (fp16)
    updc = din("updc", [P, 1], F32)       # true_u at pred points       (f32)

    kvw = {}
    for pre in ("k", "v"):
        kvw[pre + "12"] = din(pre + "w12", [L, H, 128, 4, M],
                              mybir.dt.float8e4)
        kvw[pre + "1c"] = din(pre + "w1c", [L, H, 4, M])
        kvw[pre + "3"] = din(pre + "w3", [L, H, 128, 2 * AD])
        kvw[pre + "b"] = din(pre + "bb", [L, H, 128, 4], mybir.dt.float32)
    vb3qd = din("vb3qd", [L, 2, 128, 1], F32)

    dsw16d = din("dsw16d", [I, D], FP16)
    dsbd = din("dsbd", [2, 128, 1], F32)
    ffw1d = din("ffw1d", [L, D, D], FP16)
    ffw2d = din("ffw2d", [L, D, D], FP16)
    ffb1d = din("ffb1d", [L, 2, 128, 1], F32)
    ffb2d = din("ffb2d", [L, 2, 128, 1], F32)
    ln1gd = din("ln1gd", [L, 2, 128, 1], F32)
    ln1bd = din("ln1bd", [L, 2, 128, 1], F32)
    ln2gd = din("ln2gd", [L, 2, 128, 1], F32)
    ln2bd = din("ln2bd", [L, 2, 128, 1], F32)
    dew1d = din("dew1d", [D, M], FP16)
    dew2d = din("dew2d", [M, M], FP16)
    dew3d = din("dew3d", [M, R], FP16)
    deb1d = din("deb1d", [2, 128, 1], F32)
    deb2d = din("deb2d", [2, 128, 1], F32)
    deb3d = din("deb3d", [1, R], F32)

    oh8d = din("oh8d", [4, 128])          # onehot head->feat-rows (bf16)
    ohtd = din("ohtd", [128, 4, R], F32)  # onehot target classes per pred

    out_d = nc.dram_tensor("out", [1, 1], F32, kind="ExternalOutput")

    with tile.TileContext(nc) as tc:
        with (
            tc.tile_pool(name="const", bufs=1) as cpool,
            tc.tile_pool(name="resident", bufs=1) as rpool,
            tc.tile_pool(name="wts", bufs=2) as wpool,
            tc.tile_pool(name="work", bufs=1) as kpool,
            tc.tile_pool(name="psum", bufs=1, space="PSUM") as pp,
        ):
            # --- constants / resident tensors ---------------------------
            ones_c128b = cpool.tile([128, 1], BF16, name="ones_c128b")
            nc.vector.memset(ones_c128b[:], 1.0)
            ones_c128f = cpool.tile([128, 1], F32, name="ones_c128f")
            nc.vector.memset(ones_c128f[:], 1.0)
            ones_rbf = cpool.tile([1, 128], F32, name="ones_rbf")
            nc.vector.memset(ones_rbf[:], 1.0)
            ones_rb16 = cpool.tile([1, 128], BF16, name="ones_rb16")
            nc.vector.memset(ones_rb16[:], 1.0)
            eps_t = cpool.tile([1, 1], F32, name="eps_t")
            nc.vector.memset(eps_t[:], 1e-5)
            nlogr_t = cpool.tile([1, 1], F32, name="nlogr_t")
            nc.vector.memset(nlogr_t[:], -float(P) * float(np.log(R)))
            oh4 = cpool.tile([4, 128], BF16, name="oh4")
            nc.sync.dma_start(oh4[:], oh8d[:])

            # u replicated at partitions 0/32/64/96 for 4-way row-packed
            # K=1 matmuls (true_u rank-1 term of mm1); issued before the
            # bulk xt transfer (first chain needs u4 + xt half 0 only)
            u4 = rpool.tile([128, V], BF16, name="u4")
            nc.sync.dma_start(
                u4.rearrange("(r c) v -> r c v", c=32)[:, 0:1, :], xt2[:])
            xt = rpool.tile([128, 2, V], FP8, name="xt")
            nc.sync.dma_start(xt[:, :, 0:1024], xtd[:, :, 0:1024])
            nc.sync.dma_start(xt[:, :, 1024:2048], xtd[:, :, 1024:2048])

            ct = rpool.tile([128, 16, P], BF16, name="ct")

            cur16 = [kpool.tile([128, P], FP16, tag=f"cur{q}", bufs=1,
                                name=f"cur{q}") for q in range(2)]

            # hoisted ff + decoder weights (resident; off the startup path)
            fw1 = [[cpool.tile([128, D], FP16, name=f"fw1_{l}{kc}")
                    for kc in range(2)] for l in range(L)]
            fw2 = [[cpool.tile([128, D], FP16, name=f"fw2_{l}{kc}")
                    for kc in range(2)] for l in range(L)]
            ffb = {}
            for l in range(L):
                for nm, _src in (("b1", ffb1d), ("b2", ffb2d)):
                    for fc in range(2):
                        ffb[(nm, l, fc)] = cpool.tile(
                            [128, 1], F32, name=f"ff{nm}{l}{fc}")
            lnw = {}
            for nm in ("ln1g", "ln1b", "ln2g", "ln2b"):
                for l in range(L):
                    for q in range(2):
                        lnw[(nm, l, q)] = cpool.tile(
                            [128, 1], F32, name=f"{nm}{l}{q}")
            vb3t = {}
            for l in range(L):
                for q in range(2):
                    vb3t[(l, q)] = cpool.tile([128, 1], F32,
                                              name=f"vb3t{l}{q}")
            dw1 = [cpool.tile([128, M], FP16, name=f"dw1_{kc}")
                   for kc in range(2)]
            dw2 = [cpool.tile([128, M], FP16, name=f"dw2_{kc}")
                   for kc in range(2)]
            dw3 = [cpool.tile([128, R], FP16, name=f"dw3_{kc}")
                   for kc in range(2)]
            deb1 = [cpool.tile([128, 1], F32, name=f"deb1_{fc}")
                    for fc in range(2)]
            deb2 = [cpool.tile([128, 1], F32, name=f"deb2_{fc}")
                    for fc in range(2)]
            db3f = cpool.tile([1, R], F32, name="db3f")

            def ct_piece(c0, c1):
                # ct[p, c, q] = ctm[c*128 + p, q]; contiguous 128-row blocks
                for c in range(c0, c1):
                    nc.sync.dma_start(ct[:, c, :],
                                      ctm[128 * c:128 * (c + 1), :])

            def hoist_ln():
                lnsrc = {"ln1g": ln1gd, "ln1b": ln1bd,
                         "ln2g": ln2gd, "ln2b": ln2bd}
                for nm in ("ln1g", "ln1b", "ln2g", "ln2b"):
                    for l in range(L):
                        for q in range(2):
                            nc.sync.dma_start(
                                lnw[(nm, l, q)][:], lnsrc[nm][l, q, :, :])

            def hoist_ffb():
                for l in range(L):
                    for fc in range(2):
                        nc.sync.dma_start(ffb[("b1", l, fc)][:],
                                          ffb1d[l, fc, :, :])
                        nc.sync.dma_start(ffb[("b2", l, fc)][:],
                                          ffb2d[l, fc, :, :])
                        nc.sync.dma_start(vb3t[(l, fc)][:],
                                          vb3qd[l, fc, :, :])

            def hoist_fw(l):
                for kc in range(2):
                    sl = slice(128 * kc, 128 * (kc + 1))
                    nc.sync.dma_start(fw1[l][kc][:], ffw1d[l, sl, :])
                    nc.sync.dma_start(fw2[l][kc][:], ffw2d[l, sl, :])

            def hoist_dw():
                for kc in range(2):
                    sl = slice(128 * kc, 128 * (kc + 1))
                    nc.sync.dma_start(dw1[kc][:], dew1d[sl, :])
                    nc.sync.dma_start(dw2[kc][:], dew2d[sl, :])
                    nc.sync.dma_start(dw3[kc][:], dew3d[sl, :])
                for fc in range(2):
                    nc.sync.dma_start(deb1[fc][:], deb1d[fc, :, :])
                    nc.sync.dma_start(deb2[fc][:], deb2d[fc, :, :])
                nc.sync.dma_start(db3f[:], deb3d[:])

            hoists = [
                lambda: ct_piece(0, 2),
                lambda: (ct_piece(2, 4), hoist_ln()),
                lambda: (ct_piece(4, 6), hoist_ffb()),
                lambda: (ct_piece(6, 8), hoist_fw(0)),
                lambda: (ct_piece(8, 10), hoist_fw(1)),
                lambda: (ct_piece(10, 12), hoist_dw()),
                lambda: ct_piece(12, 16),
            ]

            # keys (transposed, per (l, quad)) and vals (+ones, per (l,h))
            kt = [[rpool.tile([128, V], BF16, name=f"kt{l}{q}")
                   for q in range(2)] for l in range(L)]
            vals = [[rpool.tile([128, 16, AD + 1], BF16, name=f"vals{l}{h}")
                     for h in range(H)] for l in range(L)]

            def big_tile(nm):
                return pp.tile([128, 1024], mybir.dt.float32, tag="big",
                               name=nm, bufs=3, uniquify=True)

            def psA_tile(nm):
                return pp.tile([128, 512], mybir.dt.float32, tag="psA",
                               name=nm, bufs=2, uniquify=True)

            # elementwise engine assignment: 3/5 ACT, 2/5 DVE (DVE also
            # carries the count-multiply + LN chains)
            ew_state = [0]
            ew_pat = [(1, 0)]

            def ew_next():
                pat = ew_pat[0]
                s = ew_state[0]
                ew_state[0] = (s + 1) % len(pat)
                return pat[s % len(pat)]

            def relu_out(dst, src_ps, bias):
                """dst = relu(src_ps + bias), alternating ACT/DVE."""
                if ew_next():
                    nc.scalar.activation(dst, src_ps, AF.Relu, bias=bias)
                else:
                    nc.vector.tensor_scalar(dst, src_ps, bias, 0.0,
                                            ALU.add, ALU.max)

            def copy_out(dst, src_ps):
                """dst = src_ps (psum evacuate), alternating ACT/DVE."""
                if ew_next():
                    nc.scalar.activation(dst, src_ps, AF.Identity)
                else:
                    nc.vector.tensor_copy(dst, src_ps)

            # =============================================================
            # Phase helpers (generators yield at interleave boundaries)
            # =============================================================
            def kv_chain(l, h, pre):
                """One (layer, head, k-or-v) MLP chain over all V rows."""
                cn = f"{pre}{l}{h}"
                w12 = wpool.tile([128, 4, M], FP8, tag="w12",
                                 name=f"w12{cn}")
                w1c4 = wpool.tile([128, M], BF16, tag="w1c4", name=f"w1c4{cn}")
                w3p = wpool.tile([128, 2 * AD], BF16, tag="w3p",
                                 name=f"w3p{cn}")
                bdt = wpool.tile([128, 4], mybir.dt.float32, tag="bdt",
                                 name=f"bdt{cn}")
                # 2 sync-issued + 2 ACT-issued DMAs (DMA descriptor issue is
                # ~600ns of serial sequencer time -- 8 per chain paced the
                # whole chain phase)
                nc.sync.dma_start(w12[:], kvw[pre + "12"][l, h])
                nc.sync.dma_start(
                    w1c4.rearrange("(r c) m -> r c m", c=32)[:, 0:1, :],
                    kvw[pre + "1c"][l, h])
                nc.scalar.dma_start(w3p[:], kvw[pre + "3"][l, h])
                nc.scalar.dma_start(bdt[:], kvw[pre + "b"][l, h])
                w1 = w12[:, 0:2, :]
                w2 = w12[:, 2:4, :]
                w3a = w3p[:, 0:AD]
                w3b = w3p[:, AD:2 * AD]
                b1 = [bdt[:, fc:fc + 1] for fc in range(2)]
                b2 = [bdt[:, 2 + fc:3 + fc] for fc in range(2)]

                h18 = kpool.tile([128, 2, V], FP8, tag="h18",
                                 name=f"h18{cn}", bufs=3, uniquify=True)
                h2t = {}
                for ntp in range(2):
                    hsl = slice(1024 * ntp, 1024 * (ntp + 1))
                    # one psum alloc per fc half-wave (deeper ring pipelining)
                    for fc in range(2):
                        ps1 = big_tile(f"ps1{cn}{fc}{ntp}")
                        cs = slice(128 * fc, 128 * (fc + 1))
                        # rank-1 true_u term: K=1 bf16, row-packed at
                        # rows 64*fc + 32*j -> concurrent PE sub-arrays
                        for j in range(2):
                            sl = slice(1024 * ntp + 512 * j,
                                       1024 * ntp + 512 * (j + 1))
                            col = slice(512 * j, 512 * (j + 1))
                            row = 64 * fc + 32 * j
                            nc.tensor.matmul(
                                ps1[:, col],
                                w1c4[row:row + 1, 128 * fc:128 * (fc + 1)],
                                u4[row:row + 1, sl],
                                start=True, stop=False,
                                tile_position=(row, 0))
                        for j in range(2):
                            sl = slice(1024 * ntp + 512 * j,
                                       1024 * ntp + 512 * (j + 1))
                            col = slice(512 * j, 512 * (j + 1))
                            nc.tensor.matmul(
                                ps1[:, col], w1[:, :, cs],
                                xt[:, :, sl],
                                start=False, stop=True, perf_mode=DR)
                        relu_out(h18[:, fc, hsl], ps1[:], b1[fc])
                    yield
                for ntp in range(2):
                    for fc in range(2):
                        ps2 = big_tile(f"ps2{cn}{fc}{ntp}")
                        cs = slice(128 * fc, 128 * (fc + 1))
                        for j in range(2):
                            col = slice(512 * j, 512 * (j + 1))
                            sl = slice(1024 * ntp + 512 * j,
                                       1024 * ntp + 512 * (j + 1))
                            nc.tensor.matmul(ps2[:, col], w2[:, :, cs],
                                             h18[:, :, sl],
                                             start=True, stop=True,
                                             perf_mode=DR)
                        t = kpool.tile([128, 1024], BF16, tag="h2",
                                       name=f"h2{cn}{fc}{ntp}", bufs=6,
                                       uniquify=True)
                        relu_out(t[:], ps2[:], b2[fc])
                        h2t[(fc, ntp)] = t
                    yield

                if pre == "k":
                    q, hp = h // 4, h % 4
                    # 4-way column-packed: col tile s covers keys quarter s
                    psk = big_tile(f"psk{cn}")
                    # stationary-major order: 4 col-packed w3a matmuls, then
                    # 4 col-packed w3b (shared stationary, concurrent cols)
                    for s in range(4):
                        ntp, j = s // 2, s % 2
                        col = slice(512 * j, 512 * (j + 1))
                        nc.tensor.matmul(psk[32 * s:32 * (s + 1), 0:512],
                                         w3a, h2t[(0, ntp)][:, col],
                                         start=True, stop=False,
                                         tile_position=(0, 32 * s))
                    for s in range(4):
                        ntp, j = s // 2, s % 2
                        col = slice(512 * j, 512 * (j + 1))
                        nc.tensor.matmul(psk[32 * s:32 * (s + 1), 0:512],
                                         w3b, h2t[(1, ntp)][:, col],
                                         start=False, stop=True,
                                         tile_position=(0, 32 * s))
                    # kb3 cancels in softmax (constant per (p,h)) -> no bias
                    for s in range(4):
                        dst = kt[l][q][32 * hp:32 * (hp + 1),
                                       512 * s:512 * (s + 1)]
                        copy_out(dst, psk[32 * s:32 * (s + 1), 0:512])
                else:
                    psv = big_tile(f"psv{cn}")
                    for svg in range(16):
                        ntp, w = svg // 8, svg % 8
                        j, c = w // 4, w % 4
                        vsl = slice(512 * j + 128 * c,
                                    512 * j + 128 * (c + 1))
                        osl = slice(32 * svg, 32 * (svg + 1))
                        nc.tensor.matmul(
                            psv[:, osl], h2t[(0, ntp)][:, vsl], w3a,
                            start=(svg == 0), stop=False)
                        nc.tensor.matmul(
                            psv[:, osl], h2t[(1, ntp)][:, vsl], w3b,
                            start=False, stop=(svg == 15))
                    vt = vals[l][h]
                    nc.vector.tensor_copy(
                        vt[:, :, 0:AD],
                        psv[:, 0:512].rearrange("p (s d) -> p s d", d=AD))
                    nc.vector.memset(vt[:, :, AD:AD + 1], 1.0)
                yield

            def split_bf(src_tiles, tagp, need_lo=True):
                """f32 [128,P] tiles -> (hi bf16, lo bf16) tiles."""
                his, los = [], []
                for q, s in enumerate(src_tiles):
                    hi = kpool.tile([128, P], BF16, tag=f"{tagp}h{q}",
                                    name=f"{tagp}h{q}", bufs=2, uniquify=True)
                    nc.vector.tensor_copy(hi[:], s[:])
                    his.append(hi)
                    if need_lo:
                        lo = kpool.tile([128, P], BF16, tag=f"{tagp}l{q}",
                                        name=f"{tagp}l{q}", bufs=2,
                                        uniquify=True)
                        nc.vector.tensor_tensor(lo[:], s[:], hi[:],
                                                ALU.subtract)
                        los.append(lo)
                return his, los

            def layer_norm(xq, pfx, l, nm, box):
                """T-layout LN over 256 features (generator; appends two
                f32 tiles to box)."""
                xh, _ = split_bf(xq, "lnx", need_lo=False)
                psum2 = big_tile(f"lnsums{nm}")
                pst = psum2[0:1, 0:512]
                psq = psum2[0:1, 512:1024]
                nc.tensor.matmul(pst, ones_c128b[:], xh[0][:],
                                 start=True, stop=False)
                nc.tensor.matmul(pst, ones_c128b[:], xh[1][:],
                                 start=False, stop=True)
                sq = [kpool.tile([128, P], BF16, tag=f"lnsq{q}",
                                 name=f"lnsq{nm}{q}", bufs=1)
                      for q in range(2)]
                for q in range(2):
                    nc.vector.tensor_tensor(sq[q][:], xh[q][:], xh[q][:],
                                            ALU.mult)
                nc.tensor.matmul(psq, ones_c128b[:], sq[0][:],
                                 start=True, stop=False)
                nc.tensor.matmul(psq, ones_c128b[:], sq[1][:],
                                 start=False, stop=True)
                yield
                mu = kpool.tile([1, P], mybir.dt.float32, tag="lnmu", bufs=1,
                                name=f"lnmu{nm}")
                nc.scalar.mul(mu[:], pst, 1.0 / D)
                m2 = kpool.tile([1, P], mybir.dt.float32, tag="lnm2", bufs=1,
                                name=f"lnm2{nm}")
                nc.vector.tensor_tensor(m2[:], mu[:], mu[:], ALU.mult)
                var = kpool.tile([1, P], mybir.dt.float32, tag="lnvar",
                                 bufs=1, name=f"lnvar{nm}")
                nc.vector.scalar_tensor_tensor(
                    var[:], psq, 1.0 / D, m2[:], ALU.mult, ALU.subtract)
                # rstd = exp(-0.5*ln(var+eps)) -- Ln/Exp share one ACT table
                # set (no Sqrt-set reload, no slow DVE reciprocal)
                lnv = kpool.tile([1, P], mybir.dt.float32, tag="lnsd", bufs=1,
                                 name=f"lnsd{nm}")
                nc.scalar.activation(lnv[:], var[:], AF.Ln, bias=eps_t[:])
                rstd = kpool.tile([1, P], mybir.dt.float32, tag="lnrs",
                                  bufs=1, name=f"lnrs{nm}")
                nc.scalar.activation(rstd[:], lnv[:], AF.Exp, scale=-0.5)
                nmu = kpool.tile([1, P], mybir.dt.float32, tag="lnnm", bufs=1,
                                 name=f"lnnm{nm}")
                nc.vector.scalar_tensor_tensor(
                    nmu[:], mu[:], -1.0, rstd[:], ALU.mult, ALU.mult)
                # bf16 copies: the broadcast matmuls run at full rate (f32
                # matmuls are half-rate with slow LDWs)
                rsb = kpool.tile([1, 2, P], BF16, tag="lnrsb", bufs=1,
                                 name=f"lnrsb{nm}")
                nc.vector.tensor_copy(rsb[:, 0, :], rstd[:])
                nc.vector.tensor_copy(rsb[:, 1, :], nmu[:])
                yield
                # broadcast A=rstd, B=-mu*rstd to 128 partitions
                psab = big_tile(f"lnAB{nm}")
                nc.tensor.matmul(psab[:, 0:512], ones_rb16[:],
                                 rsb[:, 0, :], start=True, stop=True)
                nc.tensor.matmul(psab[:, 512:1024], ones_rb16[:],
                                 rsb[:, 1, :], start=True, stop=True)
                for q in range(2):
                    g = lnw[(pfx + "g", l, q)]
                    bb = lnw[(pfx + "b", l, q)]
                    t1 = kpool.tile([128, P], mybir.dt.float32, tag=f"lnt{q}",
                                    name=f"lnt{nm}{q}", bufs=1)
                    nc.vector.tensor_tensor(t1[:], xq[q][:], psab[:, 0:512],
                                            ALU.mult)
                    nc.vector.tensor_tensor(t1[:], t1[:], psab[:, 512:1024],
                                            ALU.add)
                    o = kpool.tile([128, P], mybir.dt.float32, tag=f"attv{q}",
                                   name=f"ln_out{nm}{q}", bufs=2)
                    nc.vector.tensor_scalar(o[:], t1[:], g[:], bb[:],
                                            ALU.mult, ALU.add)
                    box.append(o)
                yield

            def attention(l, attv, out_box, av_lag=0):
                """Count-matrix softmax attention (generator; yields per vc).
                4-way row-packed QK per quad; appends LN output to out_box.
                av_lag delays AV emission so early steps need no vals yet."""
                qt_h, _ = split_bf(attv, "qt", need_lo=False)
                numer = [kpool.tile([128, P], mybir.dt.float32, tag=f"num{q}",
                                    name=f"numer{l}{q}", bufs=1)
                         for q in range(2)]
                xres = []

                def ep_tail(q, rds):
                    """recip broadcast + residual for quad q -- deferred so
                    PE work can be emitted in between.  The four per-head
                    recip rows broadcast via col-packed rank-1 matmuls."""
                    psrb = big_tile(f"psrb{l}{q}")
                    for i in range(4):
                        nc.tensor.matmul(psrb[32 * i:32 * (i + 1), 0:512],
                                         ones_rb16[:, 0:32], rds[i][:],
                                         start=True, stop=True,
                                         tile_position=(0, 32 * i))
                    t1 = kpool.tile([128, P], mybir.dt.float32,
                                    tag=f"xres{q}", name=f"xres{l}{q}",
                                    bufs=1)
                    nc.vector.tensor_tensor(t1[:], numer[q][:],
                                            psrb[:, 0:512], ALU.mult)
                    # + attv (residual) + vb3 (value-bias; softmax wts sum=1)
                    nc.vector.scalar_tensor_tensor(
                        t1[:], t1[:], vb3t[(l, q)][:], attv[q][:],
                        ALU.add, ALU.add)
                    xres.append(t1)

                def emit_av(q, psA, vc, es):
                    for g in range(2):
                        h0 = 4 * q + 2 * g
                        nc.tensor.matmul(
                            psA[g][0:AD + 1, :],
                            vals[l][h0][:, vc, :],
                            es[g][:, 0:512],
                            start=(vc == 0), stop=(vc == 15))
                        nc.tensor.matmul(
                            psA[g][64:64 + AD + 1, :],
                            vals[l][h0 + 1][:, vc, :],
                            es[g][:, 512:1024],
                            start=(vc == 0), stop=(vc == 15),
                            tile_position=(0, 64))

                dn4s = [None, None]
                for q in range(2):
                    psA = [psA_tile(f"psA{l}{q}{g}") for g in range(2)]
                    avq = []
                    for vc in range(16):
                        # all 4 QKs back-to-back: rows 0/32/64/96 pack
                        # concurrently in the PE sub-arrays
                        psst = [big_tile(f"pss{l}{q}{g}{vc}")
                                for g in range(2)]
                        for g in range(2):
                            for hp in (2 * g, 2 * g + 1):
                                bb = 32 * hp
                                tp = (bb, 0) if bb >= 64 else None
                                col = slice(512 * (hp % 2),
                                            512 * (hp % 2 + 1))
                                nc.tensor.matmul(
                                    psst[g][:, col],
                                    kt[l][q][bb:bb + 32,
                                             128 * vc:128 * (vc + 1)],
                                    qt_h[q][bb:bb + 32, :],
                                    start=True, stop=True, tile_position=tp)
                        es = []
                        for g in range(2):
                            e = kpool.tile([128, 1024], BF16, tag="ebuf",
                                           name=f"e{l}{q}{g}{vc}", bufs=3,
                                           uniquify=True)
                            nc.scalar.activation(e[:], psst[g][:], AF.Exp,
                                                 scale=SCALE)
                            ce = kpool.tile([128, 1024], BF16, tag="cebuf",
                                            name=f"ce{l}{q}{g}{vc}", bufs=12,
                                            uniquify=True)
                            nc.vector.tensor_tensor(
                                ce[:], e[:],
                                ct[:, vc:vc + 1, :].broadcast_to(
                                    [128, 2, P]),
                                ALU.mult)
                            es.append(ce)
                        avq.append((vc, es))
                        if len(avq) > av_lag:
                            emit_av(q, psA, *avq.pop(0))
                        yield
                        if q == 1 and vc == 2:
                            ep_tail(0, dn4s[0])
                    for vc_, es_ in avq:
                        emit_av(q, psA, vc_, es_)
                    # psA evacuation inline (frees the AV psum ring for
                    # the next quad): numerators to SBUF, denominators to
                    # per-head recip rows exp(-ln d) straight from PSUM
                    rds = []
                    for g in range(2):
                        b0, b1r = 32 * (2 * g), 32 * (2 * g + 1)
                        copy_out(numer[q][b0:b0 + 32, :], psA[g][0:32, :])
                        copy_out(numer[q][b1r:b1r + 32, :], psA[g][64:96, :])
                        for hl, prow in ((2 * g, 32), (2 * g + 1, 96)):
                            lnd = kpool.tile([1, P], mybir.dt.float32,
                                             tag="lnd1", bufs=3,
                                             name=f"lnd{l}{q}{hl}",
                                             uniquify=True)
                            nc.scalar.activation(lnd[:],
                                                 psA[g][prow:prow + 1, :],
                                                 AF.Ln)
                            rd = kpool.tile([1, P], BF16, tag="rd1", bufs=8,
                                            name=f"rd{l}{q}{hl}",
                                            uniquify=True)
                            nc.scalar.activation(rd[:], lnd[:], AF.Exp,
                                                 scale=-1.0)
                            rds.append(rd)
                    dn4s[q] = rds
                yield
                ep_tail(1, dn4s[1])
                yield
                yield from layer_norm(xres, "ln1", l, f"ln1_{l}", out_box)

            def ff_block(l, attv, out_box):
                """Feed-forward block (generator; yields between stages)."""
                av16 = []
                for kc in range(2):
                    t = kpool.tile([128, P], FP16, tag=f"ff16{kc}",
                                   name=f"ffav{l}{kc}", bufs=2)
                    nc.vector.tensor_copy(t[:], attv[kc][:])
                    av16.append(t)
                yield
                hh_t = []
                for fc in range(2):
                    psf = big_tile(f"psff1{l}{fc}")
                    for kc in range(2):
                        nc.tensor.matmul(
                            psf[:, 0:512],
                            fw1[l][kc][:, 128 * fc:128 * (fc + 1)],
                            av16[kc][:], start=(kc == 0), stop=(kc == 1))
                    th = kpool.tile([128, P], FP16, tag=f"ffhh{fc}",
                                    name=f"ffhh{l}{fc}", bufs=2)
                    relu_out(th[:], psf[:, 0:512], ffb[("b1", l, fc)][:])
                    hh_t.append(th)
                    yield
                xres2 = []
                for fc in range(2):
                    psf2 = big_tile(f"psff2{l}{fc}")
                    for kc in range(2):
                        nc.tensor.matmul(
                            psf2[:, 0:512],
                            fw2[l][kc][:, 128 * fc:128 * (fc + 1)],
                            hh_t[kc][:], start=(kc == 0), stop=(kc == 1))
                    t2 = kpool.tile([128, P], mybir.dt.float32,
                                    tag=f"xres{fc}", name=f"xr2{l}{fc}",
                                    bufs=1)
                    nc.vector.scalar_tensor_tensor(
                        t2[:], psf2[:, 0:512], ffb[("b2", l, fc)][:],
                        attv[fc][:], ALU.add, ALU.add)
                    xres2.append(t2)
                    yield
                yield from layer_norm(xres2, "ln2", l, f"ln2_{l}", out_box)

            # =============================================================
            # Emit program
            # =============================================================
            # ds projection: attv0 = cur @ ds_W + ds_b   (T-layout out);
            # deferred into the v0 phase to keep startup DMAs off the
            # first chains' critical path
            dsw16 = [cpool.tile([128, D], FP16, name=f"dsw16{kc}")
                     for kc in range(2)]
            dsb = [cpool.tile([128, 1], F32, name=f"dsb{q}")
                   for q in range(2)]
            attv = []

            def emit_ds():
                for kc in range(2):
                    nc.sync.dma_start(dsw16[kc][:],
                                      dsw16d[128 * kc:128 * (kc + 1), :])
                    nc.sync.dma_start(cur16[kc][:],
                                      cur16d[128 * kc:128 * (kc + 1), :])
                for q in range(2):
                    nc.sync.dma_start(dsb[q][:], dsbd[q, :, :])

            def emit_ds2():
                for q in range(2):
                    psd = big_tile(f"psds{q}")
                    for kc in range(2):
                        nc.tensor.matmul(
                            psd[:, 0:512],
                            dsw16[kc][:, 128 * q:128 * (q + 1)],
                            cur16[kc][:], start=(kc == 0), stop=(kc == 1))
                    o = kpool.tile([128, P], mybir.dt.float32,
                                   tag=f"attv{q}", name=f"attv0{q}", bufs=2)
                    nc.vector.tensor_scalar(o[:], psd[:, 0:512], dsb[q][:],
                                            None, ALU.add)
                    attv.append(o)

            # --- pipelined emission ------------------------------------
            def drain(g):
                for _ in g:
                    pass

            def weave_gen(gens, after=()):
                """Cross-chain pipeline (generator): chain c's mm3 tail is
                emitted right after chain c+1's first mm1 wave.  `after` is
                a sequence of (chain_idx, fn) emission hooks."""
                hooks = dict(after)
                tail = None
                for idx, g in enumerate(gens):
                    next(g)
                    yield
                    if tail is not None:
                        for _ in tail:
                            yield
                    for _ in range(3):
                        next(g)
                        yield
                    tail = g
                    if idx in hooks:
                        hooks[idx]()
                if tail is not None:
                    for _ in tail:
                        yield

            def par2(agen, asteps, cgen, ratio):
                """Advance agen by asteps yields, interleaving ~ratio chain
                steps per attention step."""
                for _ in range(asteps):
                    try:
                        next(agen)
                    except StopIteration:
                        break
                    if cgen is not None:
                        for _ in range(ratio):
                            try:
                                next(cgen)
                            except StopIteration:
                                cgen = None
                                break

            k0 = [kv_chain(0, hh, "k") for hh in range(H)]
            v0 = [kv_chain(0, hh, "v") for hh in range(H)]
            k1 = [kv_chain(1, hh, "k") for hh in range(H)]
            v1 = [kv_chain(1, hh, "v") for hh in range(H)]

            drain(weave_gen(k0[0:4], after=(
                (1, lambda: (hoists[0](), emit_ds())), (2, hoists[1]),
                (3, hoists[2]))))
            # attn0-q0's QK/exp/ce need only kt0[q0] + attv; with av_lag=6
            # its first steps overlap the v0 chains that produce vals0.
            c_head = weave_gen(v0[0:4], after=(
                (0, hoists[3]), (1, lambda: (hoists[4](), emit_ds2()))))
            for _ in range(12):
                next(c_head)
            box0 = []
            a0 = attention(0, attv, box0, av_lag=5)
            par2(a0, 6, c_head, ratio=2)
            drain(c_head)
            # q1 needs k0[4:]+v0[4:] -> fully emitted within q0's steps.
            c1a = weave_gen(k0[4:8] + v0[4:8],
                            after=((3, hoists[5]), (6, hoists[6])))
            par2(a0, 10, c1a, ratio=4)
            drain(c1a)
            # q1 + tail interleave with k1 (attn1-q0 needs all of kt1
            # emitted before attention(1) starts).
            c1b = weave_gen(k1)
            par2(a0, 21, c1b, ratio=2)
            drain(a0)
            drain(c1b)
            attv = box0
            # v1 chains fill the PE-idle ff0/attn1 windows: v1[0:4] must be
            # fully emitted before attn1's first AV (quad0), v1[4:8] before
            # its quad-1 AVs.
            c1ca = weave_gen(v1[0:4])
            box_f0 = []
            f0 = ff_block(0, attv, box_f0)
            par2(f0, 8, c1ca, ratio=3)
            drain(f0)
            drain(c1ca)
            attv = box_f0

            box1 = []
            a1 = attention(1, attv, box1, av_lag=4)
            c1cb = weave_gen(v1[4:8])
            par2(a1, 37, c1cb, ratio=2)
            drain(c1cb)
            drain(a1)
            attv = box1
            box_f1 = []
            drain(ff_block(1, attv, box_f1))
            attv = box_f1

            # ---- decoder ----------------------------------------------
            de16 = []
            for kc in range(2):
                t = kpool.tile([128, P], FP16, tag=f"de16{kc}",
                               name=f"de16{kc}", bufs=1)
                nc.vector.tensor_copy(t[:], attv[kc][:])
                de16.append(t)
            d1 = []
            for fc in range(2):
                psd1 = big_tile(f"psde1{fc}")
                for kc in range(2):
                    nc.tensor.matmul(
                        psd1[:, 0:512],
                        dw1[kc][:, 128 * fc:128 * (fc + 1)],
                        de16[kc][:], start=(kc == 0), stop=(kc == 1))
                th = kpool.tile([128, P], FP16, tag=f"d1h{fc}",
                                name=f"d1h{fc}", bufs=2)
                relu_out(th[:], psd1[:, 0:512], deb1[fc][:])
                d1.append(th)
            d2 = []
            for fc in range(2):
                psd2 = big_tile(f"psde2{fc}")
                for kc in range(2):
                    nc.tensor.matmul(
                        psd2[:, 0:512],
                        dw2[kc][:, 128 * fc:128 * (fc + 1)],
                        d1[kc][:], start=(kc == 0), stop=(kc == 1))
                th = kpool.tile([128, P], FP16, tag=f"d2h{fc}",
                                name=f"d2h{fc}", bufs=2)
                relu_out(th[:], psd2[:, 0:512], deb2[fc][:])
                d2.append(th)

            # logits row-major [p, R] per 128-p chunk + loss.  Targets are
            # host-precomputed one-hots; logits are O(10) so exp needs no
            # max-shift in f32.
            oht = kpool.tile([128, 4, R], mybir.dt.float32, tag="oht",
                             name="oht", bufs=1)
            nc.sync.dma_start(oht[:], ohtd[:])
            t4 = kpool.tile([128, 4], mybir.dt.float32, tag="t4",
                            name="t4", bufs=1)

            def logits_tail(pc, psl):
                escr = kpool.tile([128, R], mybir.dt.float32, tag="escr",
                                  bufs=2, name=f"escr{pc}")
                se = kpool.tile([128, 1], mybir.dt.float32, tag="se",
                                name=f"se{pc}", bufs=2)
                nc.scalar.activation(escr[:], psl[:, 0:R], AF.Exp,
                                     accum_out=se[:])
                ls = kpool.tile([128, 1], mybir.dt.float32, tag="ls",
                                name=f"ls{pc}", bufs=2)
                nc.scalar.activation(ls[:], se[:], AF.Ln)
                scr2 = kpool.tile([128, R], mybir.dt.float32, tag="scr2",
                                  bufs=2, name=f"scr2{pc}")
                pk = kpool.tile([128, 1], mybir.dt.float32, tag="pk",
                                name=f"pk{pc}", bufs=2)
                nc.vector.scalar_tensor_tensor(
                    scr2[:], psl[:, 0:R], 1.0, oht[:, pc, :],
                    ALU.mult, ALU.mult, accum_out=pk[:])
                nc.vector.tensor_tensor(t4[:, pc:pc + 1], pk[:], ls[:],
                                        ALU.subtract)

            prev_log = None
            for pc in range(4):
                psl = big_tile(f"pslog{pc}")
                psl_sl = slice(128 * pc, 128 * (pc + 1))
                # bias row (f32 rank-1), then the two K-chunks
                nc.tensor.matmul(psl[:, 0:R], ones_rbf[:], db3f[:],
                                 start=True, stop=False)
                for kc in range(2):
                    nc.tensor.matmul(psl[:, 0:R], d2[kc][:, psl_sl],
                                     dw3[kc][:],
                                     start=False, stop=(kc == 1))
                if prev_log is not None:
                    logits_tail(*prev_log)
                prev_log = (pc, psl)
            logits_tail(*prev_log)
            pspr = big_tile("pspr")
            nc.tensor.matmul(pspr[0:1, 0:4], ones_c128f[:], t4[:],
                             start=True, stop=True)
            pr4 = kpool.tile([1, 4], mybir.dt.float32, tag="pr4",
                             name="pr4", bufs=1)
            nc.vector.tensor_copy(pr4[:], pspr[0:1, 0:4])
            s1 = kpool.tile([1, 1], mybir.dt.float32, tag="s1",
                            name="s1", bufs=1)
            nc.vector.tensor_reduce(s1[:], pr4[:], mybir.AxisListType.X,
                                    ALU.add)
            outt = kpool.tile([1, 1], mybir.dt.float32, tag="outt",
                              name="outt", bufs=1)
            nc.scalar.activation(outt[:], s1[:], AF.Identity,
                                 bias=nlogr_t[:], scale=-1.0)
            nc.sync.dma_start(out_d[:], outt[:])

    return nc


F8 = getattr(ml_dtypes, "float8_e4m3", ml_dtypes.float8_e4m3fn)


def _kpack(w, dt=None):
    """[256, F] -> K-halves packed [128, 2, F]."""
    w = np.asarray(w, np.float32)
    return np.ascontiguousarray(
        w.reshape(2, 128, w.shape[-1]).transpose(1, 0, 2)).astype(dt or F8)


def _maybe_enable_trace():
    """Optional NTFF profiling under axon (KERNEL_TRACE=1); best-effort."""
    try:
        import sys
        import types

        import antenv

        if "antenv.axon_hooks" not in sys.modules:
            mod = types.ModuleType("antenv.axon_hooks")
            mod._hook = None
            mod.set_axon_ntff_profile_hook = lambda h: setattr(mod, "_hook", h)
            mod.get_axon_ntff_profile_hook = lambda: mod._hook
            sys.modules["antenv.axon_hooks"] = mod
            antenv.axon_hooks = mod
            from trn_agent_boot.trn_boot import _ntff_profile_via_ctypes

            mod._hook = _ntff_profile_via_ctypes("/opt/axon/libaxon_pjrt.so")
        import concourse.bass_utils as _bu

        _bu.upload_artifacts = lambda tmpdir: f"file://{tmpdir}"
        return True
    except Exception:
        return False


LAST_RESULT = {}


def _enable_ldw_opt():
    """Turn on walrus LDWEIGHTS dedup (consecutive matmuls sharing a
    stationary tile skip the reload) for our own compile invocation."""
    import concourse.bass_utils as _bu

    if getattr(_bu, "_ldw_opt_patched", False):
        return
    orig = _bu.run_command

    # ldw-opt errors out on tile_position LDWs ("not compatible with LDW
    # optimization") -- left disabled.
    _ = orig
    _bu._ldw_opt_patched = True


def kernel(**inputs):
    from concourse.bass_utils import run_bass_kernel_spmd
    _enable_ldw_opt()

    if "nc" not in _BUILT:
        _BUILT["nc"] = _build()
    nc = _BUILT["nc"]

    f32 = lambda a: np.ascontiguousarray(np.asarray(a, np.float32))
    bf = lambda a: np.ascontiguousarray(np.asarray(a, np.float32)).astype(BF)
    f16 = lambda a: np.ascontiguousarray(
        np.asarray(a, np.float32)).astype(np.float16)

    enc = f32(inputs["encoded"])                      # [B,V,I]
    tu = f32(inputs["true_u"])                        # [B,V,1]
    mask = f32(inputs["attn_mask"])                   # [P,N]
    pp_ = np.asarray(inputs["pred_points"]).astype(np.int64)
    ni = np.asarray(inputs["neighbor_index"]).astype(np.int64)

    # count matrix C[p, v]
    C = np.zeros((P, V), np.float32)
    np.add.at(C, (np.repeat(np.arange(P), N), ni.ravel()),
              np.exp(-SCALE * mask).ravel().astype(np.float32))
    ctm = np.ascontiguousarray(C.T).astype(BF)        # [V, P]

    shared = {"ctm": ctm}
    for pre in ("k", "v"):
        w1 = f32(inputs[pre + "W1"])                  # [L,H,257,256]
        w2 = f32(inputs[pre + "W2"])
        shared[pre + "w12"] = np.stack(
            [np.stack([np.concatenate(
                [_kpack(w1[l, h, 0:256]), _kpack(w2[l, h])], axis=1)
                for h in range(H)]) for l in range(L)])
        # u rank-1 row replicated 4x for 4-way row packing
        shared[pre + "1c"] = np.ascontiguousarray(np.broadcast_to(
            w1[:, :, 256:257, :], (L, H, 4, M))).astype(BF)
        w3 = f32(inputs[pre + "W3"])                  # [L,H,256,32]
        shared[pre + "w3"] = np.concatenate(
            [w3[:, :, 0:128, :], w3[:, :, 128:256, :]], axis=3).astype(BF)
        b1 = f32(inputs[pre + "b1"]).reshape(L, H, 2, 128)
        b2 = f32(inputs[pre + "b2"]).reshape(L, H, 2, 128)
        shared[pre + "bb"] = np.ascontiguousarray(
            np.stack([b1[:, :, 0], b1[:, :, 1], b2[:, :, 0], b2[:, :, 1]],
                     axis=3))
    # rename to match din names
    shared["kw1c"] = shared.pop("k1c")
    shared["vw1c"] = shared.pop("v1c")
    # vb3 folded into residual (softmax weights sum to 1); kb3 cancels.
    shared["vb3qd"] = np.ascontiguousarray(
        f32(inputs["vb3"]).reshape(L, 2, 128)[:, :, :, None])

    shared["dsw16d"] = f16(inputs["ds_W"])
    shared["dsbd"] = f32(inputs["ds_b"]).reshape(2, 128, 1)
    shared["ffw1d"] = f16(inputs["ff_W1"])
    shared["ffw2d"] = f16(inputs["ff_W2"])
    shared["ffb1d"] = f32(inputs["ff_b1"]).reshape(L, 2, 128, 1)
    shared["ffb2d"] = f32(inputs["ff_b2"]).reshape(L, 2, 128, 1)
    shared["ln1gd"] = f32(inputs["ln1_g"]).reshape(L, 2, 128, 1)
    shared["ln1bd"] = f32(inputs["ln1_b"]).reshape(L, 2, 128, 1)
    shared["ln2gd"] = f32(inputs["ln2_g"]).reshape(L, 2, 128, 1)
    shared["ln2bd"] = f32(inputs["ln2_b"]).reshape(L, 2, 128, 1)
    shared["dew1d"] = f16(inputs["de_W1"])
    shared["dew2d"] = f16(inputs["de_W2"])
    shared["dew3d"] = f16(inputs["de_W3"])
    shared["deb1d"] = f32(inputs["de_b1"]).reshape(2, 128, 1)
    shared["deb2d"] = f32(inputs["de_b2"]).reshape(2, 128, 1)
    shared["deb3d"] = f32(inputs["de_b3"]).reshape(1, R)

    oh4f = np.zeros((4, 128), np.float32)
    for i in range(4):
        oh4f[i, 32 * i:32 * (i + 1)] = 1.0
    shared["oh8d"] = oh4f.astype(BF)

    in_maps = []
    for b in range(B):
        merged = np.concatenate([enc[b], tu[b]], axis=1)  # [V, 257]
        mt = np.ascontiguousarray(merged.T)               # [257, V]
        cur = enc[b][pp_, :]                              # [P, I]
        curt = np.ascontiguousarray(cur.T)                # [I, P]
        m = dict(shared)
        m["xtd"] = _kpack(mt[0:256])
        m["xt2"] = np.ascontiguousarray(
            np.broadcast_to(mt[256:257], (4, V))).astype(BF)
        m["cur16d"] = curt.astype(np.float16)
        m["updc"] = tu[b][pp_, :]                          # [P,1] f32
        tgt = np.clip(np.floor(tu[b][pp_, 0] * R).astype(np.int64), 0, R - 1)
        oht = np.zeros((P, R), np.float32)
        oht[np.arange(P), tgt] = 1.0
        # oht[p_local, pc, r]
        m["ohtd"] = np.ascontiguousarray(
            oht.reshape(4, 128, R).transpose(1, 0, 2))
        in_maps.append(m)

    trace = os.environ.get("KERNEL_TRACE") == "1" and _maybe_enable_trace()
    res = run_bass_kernel_spmd(
        nc, in_maps, core_ids=list(range(B)), trace=trace,
        trace_cores=list(range(B)) if trace else None)
    LAST_RESULT["res"] = res
    if trace and res.exec_time_ns is not None:
        print(f"HW exec time: {res.exec_time_ns} ns "
              f"(mean {res.mean_exec_time_ns} ns, "
              f"slowest core {res.max_exec_time_core_id})")
    out = np.array([res.results[b]["out"][0, 0] for b in range(B)], np.float32)
    return out


# revision 69
# speedup vs baseline: 1.0336x; 1.0134x over previous
"""Trainium2 Bass kernel for nn_CopulaDecoder.  (HW: ~508us, rel err 3.7e-3)

Data-parallel over batch: core b computes batch element b end-to-end.
All activations live transposed (features on partitions, tokens on free dim).
The neighbor-gather softmax is reformulated as a dense count-matrix softmax:
  softmax over the 64 gathered scores == (C * exp(scale*S)) normalized, where
  C[p,v] = sum_n 1[neighbor_index[p,n]==v] * exp(-scale*attn_mask[p,n]).
Scores are small (|scale*S| < ~4 for this model family), so no max-shift.

Bias algebra: the key-MLP output bias kb3 shifts every score for a given
(p, head) by the same constant q.kb3 -> cancels in softmax -> dropped.
The value-MLP output bias vb3 shifts the attention output by vb3 (softmax
weights sum to 1) -> folded into the residual add before LN1.  Target
one-hots for the final NLL are host-precomputed; logits are O(10) so the
log-softmax needs no max-shift.  All reciprocals (LN rstd, softmax denom)
are exp(-ln x) / exp(-0.5 ln(x+eps)) on ACT: Ln+Exp live in one activation
table set (natural_log_exp), so the kernel never reloads ACT tables and
avoids the 8-cycle/elem DVE divide.

Precision: fp8-e4m3 DoubleRow (K=256/pass) for the two big KV-MLP matmuls,
bf16 for mm3 + attention, fp16 single for the small matmuls (ds/ff/decoder)
with f32 per-partition biases applied in the epilogue ops (no bias_mm
rank-1s, no hi/lo splits), fp32 accumulate, fp32 elementwise.

Scheduling: one software pipeline across the whole net.  Emission order ==
engine queue order (in-order engines), so overlap is achieved by
interleaving emission subject to producer-before-consumer order:
  k0[0:4] -> [attn0-q0 (av_lag) || v0] -> [attn0 || k0/v0 tails, k1] ->
  [ff0 || v1[0:4]] -> [attn1 || v1[4:8]] -> ff1 -> decoder (pipelined
  logits).  DMA hoists (count matrix in 16 pieces, weights) are spread
  across the head chains so no transfer ever blocks chain weights.  DMA
  descriptor issue costs ~600ns of serial sequencer time, so chain weights
  arrive in 2 sync-issued DMAs (w1+w2 packed into one fp8 tensor; the 4
  rank-1 rows as one partition-strided transfer) + 2 ACT-issued ones.
attention defers AV emission by av_lag steps (deep ce ring) so its
QK/exp/ce can overlap chains that produce the values; the per-quad
denominator pipeline (psA evacuate -> recip -> broadcast -> residual) is
deferred into the next quad's steps to avoid head-of-line PE stalls.
The true_u rank-1 (K=1) matmuls are row-packed pairs (rows 64fc+32j) that
run concurrently in the PE sub-arrays; QK is 4-way row-packed with all
four matmuls emitted back-to-back; AV/mm3-k use column packing.
PSUM: one [128,1024] ring (bufs=3) shared by chains/scores/misc + a
[128,512] AV ring (bufs=2).
"""
import os

import numpy as np
import ml_dtypes

B, S, T = 8, 32, 64
V = S * T
P = 512
N = 2 * S
I = 256
H, AD = 8, 32
D = H * AD
M = 256
L = 2
R = 128
SCALE = float(AD) ** -0.5

BF = ml_dtypes.bfloat16

_BUILT = {}


# ---------------------------------------------------------------------------
# walrus wait-slot workaround (inlined; see dev notes): Tile attaches >1
# semaphore wait to one instruction; many ISA encodings have a single wait
# slot.  Peel excess waits onto injected same-engine InstNoOps.
# ---------------------------------------------------------------------------
def _install_waitfix():
    import bass_rust
    import concourse.mybir as mybir
    import concourse.tile as tile_mod

    if getattr(tile_mod.TileContext, "_waitfix_installed", False):
        return
    limits = {"InstDrain": 1000, "InstEventSemaphore": 1000, "InstCall": 1000,
              "InstISA": 0}
    counter = [0]
    orig_add = tile_mod.TileContext._add_instruction

    def patched_add(self, inst):
        si = inst.sync_info
        if si is not None:
            limit = limits.get(type(inst).__name__, 1)
            waits = list(si.on_wait)
            if len(waits) > limit:
                keep = waits[-limit:] if limit else []
                excess = waits[:-limit] if limit else waits
                while excess:
                    chunk, excess = excess[:1], excess[1:]
                    counter[0] += 1
                    nop = bass_rust.InstNoOp(
                        name=f"waitsplit-{counter[0]}", ins=[], outs=[])
                    nop.engine = inst.engine
                    nop.sync_info = mybir.SyncInfo(on_wait=chunk, on_update=[])
                    orig_add(self, nop)
                inst.sync_info = mybir.SyncInfo(
                    on_wait=keep, on_update=list(si.on_update))
        orig_add(self, inst)

    def patched_drain_and_barrier(self, tick_clock, wait_clock):
        from concourse.tile import ScopedClock

        drain_inst = self.nc.sync.drain()
        wait_clock.add_sem_waits(
            drain_inst.ins, ScopedClock({None: tick_clock.global_clock}))
        si = drain_inst.ins.sync_info
        if si is not None and len(si.on_wait) > 1:
            waits = list(si.on_wait)
            drain_inst.ins.sync_info = mybir.SyncInfo(
                on_wait=waits[:1], on_update=list(si.on_update))
            rest = waits[1:]
            while rest:
                chunk, rest = rest[:1], rest[1:]
                nop = self.nc.sync.nop()
                nop.ins.sync_info = mybir.SyncInfo(on_wait=chunk, on_update=[])
        self.nc.all_engine_barrier()
        assert self.sems is not None
        popped = self.nc._tile_sem_poison_stack.pop()
        assert popped is self._sem_poison
        self.nc.clear_and_free_semaphores(list(self.sems.allocated().values()))
        self.nc.all_engine_barrier()

    try:
        import concourse.tile_utils as tile_utils
        tile_utils.max_sbuf_usage = 204 * 1024
    except Exception:
        pass
    tile_mod.TileContext._add_instruction = patched_add
    tile_mod.TileContext._drain_and_barrier = patched_drain_and_barrier
    tile_mod.TileContext._waitfix_installed = True


def _build():
    """Emit the single-core Bass program (SPMD across 8 cores)."""
    import concourse.bass as bass
    import concourse.mybir as mybir
    import concourse.tile as tile

    _install_waitfix()

    F32 = mybir.dt.float32
    BF16 = mybir.dt.bfloat16
    FP16 = mybir.dt.float16
    FP8 = mybir.dt.float8e4
    DR = mybir.MatmulPerfMode.DoubleRow
    AF = mybir.ActivationFunctionType
    ALU = mybir.AluOpType

    nc = bass.Bass()

    def din(name, shape, dt=BF16):
        return nc.dram_tensor(name, list(shape), dt, kind="ExternalInput")

    # --- DRAM inputs -------------------------------------------------------
    xtd = din("xtd", [128, 2, V], mybir.dt.float8e4)  # merged.T rows 0:256
    xt2 = din("xt2", [4, V])              # row 256 (true_u)            (bf16)
    ctm = din("ctm", [V, P])              # count matrix transposed     (bf16)
    cur16d = din("cur16d", [I, P], FP16)  # cur.T                       (fp16)
    updc = din("updc", [P, 1], F32)       # true_u at pred points       (f32)

    kvw = {}
    for pre in ("k", "v"):
        kvw[pre + "12"] = din(pre + "w12", [L, H, 128, 4, M],
                              mybir.dt.float8e4)
        kvw[pre + "1c"] = din(pre + "w1c", [L, H, 4, M])
        kvw[pre + "3"] = din(pre + "w3", [L, H, 128, 2 * AD])
        kvw[pre + "b"] = din(pre + "bb", [L, H, 128, 4], mybir.dt.float32)
    vb3qd = din("vb3qd", [L, 2, 128, 1], F32)

    dsw16d = din("dsw16d", [I, D], FP16)
    dsbd = din("dsbd", [2, 128, 1], F32)
    ffw1d = din("ffw1d", [L, D, D], FP16)
    ffw2d = din("ffw2d", [L, D, D], FP16)
    ffb1d = din("ffb1d", [L, 2, 128, 1], F32)
    ffb2d = din("ffb2d", [L, 2, 128, 1], F32)
    ln1gd = din("ln1gd", [L, 2, 128, 1], F32)
    ln1bd = din("ln1bd", [L, 2, 128, 1], F32)
    ln2gd = din("ln2gd", [L, 2, 128, 1], F32)
    ln2bd = din("ln2bd", [L, 2, 128, 1], F32)
    dew1d = din("dew1d", [D, M], FP16)
    dew2d = din("dew2d", [M, M], FP16)
    dew3d = din("dew3d", [M, R], FP16)
    deb1d = din("deb1d", [2, 128, 1], F32)
    deb2d = din("deb2d", [2, 128, 1], F32)
    deb3d = din("deb3d", [1, R], F32)

    oh8d = din("oh8d", [4, 128])          # onehot head->feat-rows (bf16)
    ohtd = din("ohtd", [128, 4, R], F32)  # onehot target classes per pred

    out_d = nc.dram_tensor("out", [1, 1], F32, kind="ExternalOutput")

    with tile.TileContext(nc) as tc:
        with (
            tc.tile_pool(name="const", bufs=1) as cpool,
            tc.tile_pool(name="resident", bufs=1) as rpool,
            tc.tile_pool(name="wts", bufs=2) as wpool,
            tc.tile_pool(name="work", bufs=1) as kpool,
            tc.tile_pool(name="psum", bufs=1, space="PSUM") as pp,
        ):
            # --- constants / resident tensors ---------------------------
            ones_c128b = cpool.tile([128, 1], BF16, name="ones_c128b")
            nc.vector.memset(ones_c128b[:], 1.0)
            ones_c128f = cpool.tile([128, 1], F32, name="ones_c128f")
            nc.vector.memset(ones_c128f[:], 1.0)
            ones_rbf = cpool.tile([1, 128], F32, name="ones_rbf")
            nc.vector.memset(ones_rbf[:], 1.0)
            ones_rb16 = cpool.tile([1, 128], BF16, name="ones_rb16")
            nc.vector.memset(ones_rb16[:], 1.0)
            eps_t = cpool.tile([1, 1], F32, name="eps_t")
            nc.vector.memset(eps_t[:], 1e-5)
            nlogr_t = cpool.tile([1, 1], F32, name="nlogr_t")
            nc.vector.memset(nlogr_t[:], -float(P) * float(np.log(R)))
            oh4 = cpool.tile([4, 128], BF16, name="oh4")
            nc.sync.dma_start(oh4[:], oh8d[:])

            # u replicated at partitions 0/32/64/96 for 4-way row-packed
            # K=1 matmuls (true_u rank-1 term of mm1); issued before the
            # bulk xt transfer (first chain needs u4 + xt half 0 only)
            u4 = rpool.tile([128, V], BF16, name="u4")
            nc.sync.dma_start(
                u4.rearrange("(r c) v -> r c v", c=32)[:, 0:1, :], xt2[:])
            xt = rpool.tile([128, 2, V], FP8, name="xt")
            nc.sync.dma_start(xt[:, :, 0:1024], xtd[:, :, 0:1024])
            nc.sync.dma_start(xt[:, :, 1024:2048], xtd[:, :, 1024:2048])

            ct = rpool.tile([128, 16, P], BF16, name="ct")

            cur16 = [kpool.tile([128, P], FP16, tag=f"cur{q}", bufs=1,
                                name=f"cur{q}") for q in range(2)]

            # hoisted ff + decoder weights (resident; off the startup path)
            fw1 = [[cpool.tile([128, D], FP16, name=f"fw1_{l}{kc}")
                    for kc in range(2)] for l in range(L)]
            fw2 = [[cpool.tile([128, D], FP16, name=f"fw2_{l}{kc}")
                    for kc in range(2)] for l in range(L)]
            ffb = {}
            for l in range(L):
                for nm, _src in (("b1", ffb1d), ("b2", ffb2d)):
                    for fc in range(2):
                        ffb[(nm, l, fc)] = cpool.tile(
                            [128, 1], F32, name=f"ff{nm}{l}{fc}")
            lnw = {}
            for nm in ("ln1g", "ln1b", "ln2g", "ln2b"):
                for l in range(L):
                    for q in range(2):
                        lnw[(nm, l, q)] = cpool.tile(
                            [128, 1], F32, name=f"{nm}{l}{q}")
            vb3t = {}
            for l in range(L):
                for q in range(2):
                    vb3t[(l, q)] = cpool.tile([128, 1], F32,
                                              name=f"vb3t{l}{q}")
            dw1 = [cpool.tile([128, M], FP16, name=f"dw1_{kc}")
                   for kc in range(2)]
            dw2 = [cpool.tile([128, M], FP16, name=f"dw2_{kc}")
                   for kc in range(2)]
            dw3 = [cpool.tile([128, R], FP16, name=f"dw3_{kc}")
                   for kc in range(2)]
            deb1 = [cpool.tile([128, 1], F32, name=f"deb1_{fc}")
                    for fc in range(2)]
            deb2 = [cpool.tile([128, 1], F32, name=f"deb2_{fc}")
                    for fc in range(2)]
            db3f = cpool.tile([1, R], F32, name="db3f")

            def ct_piece(c0, c1):
                # ct[p, c, q] = ctm[c*128 + p, q]; contiguous 128-row blocks
                for c in range(c0, c1):
                    nc.sync.dma_start(ct[:, c, :],
                                      ctm[128 * c:128 * (c + 1), :])

            def hoist_ln():
                lnsrc = {"ln1g": ln1gd, "ln1b": ln1bd,
                         "ln2g": ln2gd, "ln2b": ln2bd}
                for nm in ("ln1g", "ln1b", "ln2g", "ln2b"):
                    for l in range(L):
                        for q in range(2):
                            nc.sync.dma_start(
                                lnw[(nm, l, q)][:], lnsrc[nm][l, q, :, :])

            def hoist_ffb():
                for l in range(L):
                    for fc in range(2):
                        nc.sync.dma_start(ffb[("b1", l, fc)][:],
                                          ffb1d[l, fc, :, :])
                        nc.sync.dma_start(ffb[("b2", l, fc)][:],
                                          ffb2d[l, fc, :, :])
                        nc.sync.dma_start(vb3t[(l, fc)][:],
                                          vb3qd[l, fc, :, :])

            def hoist_fw(l):
                for kc in range(2):
                    sl = slice(128 * kc, 128 * (kc + 1))
                    nc.sync.dma_start(fw1[l][kc][:], ffw1d[l, sl, :])
                    nc.sync.dma_start(fw2[l][kc][:], ffw2d[l, sl, :])

            def hoist_dw():
                for kc in range(2):
                    sl = slice(128 * kc, 128 * (kc + 1))
                    nc.sync.dma_start(dw1[kc][:], dew1d[sl, :])
                    nc.sync.dma_start(dw2[kc][:], dew2d[sl, :])
                    nc.sync.dma_start(dw3[kc][:], dew3d[sl, :])
                for fc in range(2):
                    nc.sync.dma_start(deb1[fc][:], deb1d[fc, :, :])
                    nc.sync.dma_start(deb2[fc][:], deb2d[fc, :, :])
                nc.sync.dma_start(db3f[:], deb3d[:])

            hoists = [
                lambda: ct_piece(0, 2),
                lambda: (ct_piece(2, 4), hoist_ln()),
                lambda: (ct_piece(4, 6), hoist_ffb()),
                lambda: (ct_piece(6, 8), hoist_fw(0)),
                lambda: (ct_piece(8, 10), hoist_fw(1)),
                lambda: (ct_piece(10, 12), hoist_dw()),
                lambda: ct_piece(12, 16),
            ]

            # keys (transposed, per (l, quad)) and vals (+ones, per (l,h))
            kt = [[rpool.tile([128, V], BF16, name=f"kt{l}{q}")
                   for q in range(2)] for l in range(L)]
            vals = [[rpool.tile([128, 16, AD + 1], BF16, name=f"vals{l}{h}")
                     for h in range(H)] for l in range(L)]

            def big_tile(nm):
                return pp.tile([128, 1024], mybir.dt.float32, tag="big",
                               name=nm, bufs=3, uniquify=True)

            def psA_tile(nm):
                return pp.tile([128, 512], mybir.dt.float32, tag="psA",
                               name=nm, bufs=2, uniquify=True)

            # elementwise engine assignment: 3/5 ACT, 2/5 DVE (DVE also
            # carries the count-multiply + LN chains)
            ew_state = [0]
            ew_pat = [(1, 0)]

            def ew_next():
                pat = ew_pat[0]
                s = ew_state[0]
                ew_state[0] = (s + 1) % len(pat)
                return pat[s % len(pat)]

            def relu_out(dst, src_ps, bias):
                """dst = relu(src_ps + bias), alternating ACT/DVE."""
                if ew_next():
                    nc.scalar.activation(dst, src_ps, AF.Relu, bias=bias)
                else:
                    nc.vector.tensor_scalar(dst, src_ps, bias, 0.0,
                                            ALU.add, ALU.max)

            def copy_out(dst, src_ps):
                """dst = src_ps (psum evacuate), alternating ACT/DVE."""
                if ew_next():
                    nc.scalar.activation(dst, src_ps, AF.Identity)
                else:
                    nc.vector.tensor_copy(dst, src_ps)

            # =============================================================
            # Phase helpers (generators yield at interleave boundaries)
            # =============================================================
            def kv_chain(l, h, pre):
                """One (layer, head, k-or-v) MLP chain over all V rows."""
                cn = f"{pre}{l}{h}"
                w12 = wpool.tile([128, 4, M], FP8, tag="w12",
                                 name=f"w12{cn}")
                w1c4 = wpool.tile([128, M], BF16, tag="w1c4", name=f"w1c4{cn}")
                w3p = wpool.tile([128, 2 * AD], BF16, tag="w3p",
                                 name=f"w3p{cn}")
                bdt = wpool.tile([128, 4], mybir.dt.float32, tag="bdt",
                                 name=f"bdt{cn}")
                # 2 sync-issued + 2 ACT-issued DMAs (DMA descriptor issue is
                # ~600ns of serial sequencer time -- 8 per chain paced the
                # whole chain phase)
                nc.sync.dma_start(w12[:], kvw[pre + "12"][l, h])
                nc.sync.dma_start(
                    w1c4.rearrange("(r c) m -> r c m", c=32)[:, 0:1, :],
                    kvw[pre + "1c"][l, h])
                nc.scalar.dma_start(w3p[:], kvw[pre + "3"][l, h])
                nc.scalar.dma_start(bdt[:], kvw[pre + "b"][l, h])
                w1 = w12[:, 0:2, :]
                w2 = w12[:, 2:4, :]
                w3a = w3p[:, 0:AD]
                w3b = w3p[:, AD:2 * AD]
                b1 = [bdt[:, fc:fc + 1] for fc in range(2)]
                b2 = [bdt[:, 2 + fc:3 + fc] for fc in range(2)]

                h18 = kpool.tile([128, 2, V], FP8, tag="h18",
                                 name=f"h18{cn}", bufs=3, uniquify=True)
                h2t = {}
                for ntp in range(2):
                    hsl = slice(1024 * ntp, 1024 * (ntp + 1))
                    # one psum alloc per fc half-wave (deeper ring pipelining)
                    for fc in range(2):
                        ps1 = big_tile(f"ps1{cn}{fc}{ntp}")
                        cs = slice(128 * fc, 128 * (fc + 1))
                        # rank-1 true_u term: K=1 bf16, row-packed at
                        # rows 64*fc + 32*j -> concurrent PE sub-arrays
                        for j in range(2):
                            sl = slice(1024 * ntp + 512 * j,
                                       1024 * ntp + 512 * (j + 1))
                            col = slice(512 * j, 512 * (j + 1))
                            row = 64 * fc + 32 * j
                            nc.tensor.matmul(
                                ps1[:, col],
                                w1c4[row:row + 1, 128 * fc:128 * (fc + 1)],
                                u4[row:row + 1, sl],
                                start=True, stop=False,
                                tile_position=(row, 0))
                        for j in range(2):
                            sl = slice(1024 * ntp + 512 * j,
                                       1024 * ntp + 512 * (j + 1))
                            col = slice(512 * j, 512 * (j + 1))
                            nc.tensor.matmul(
                                ps1[:, col], w1[:, :, cs],
                                xt[:, :, sl],
                                start=False, stop=True, perf_mode=DR)
                        relu_out(h18[:, fc, hsl], ps1[:], b1[fc])
                    yield
                for ntp in range(2):
                    for fc in range(2):
                        ps2 = big_tile(f"ps2{cn}{fc}{ntp}")
                        cs = slice(128 * fc, 128 * (fc + 1))
                        for j in range(2):
                            col = slice(512 * j, 512 * (j + 1))
                            sl = slice(1024 * ntp + 512 * j,
                                       1024 * ntp + 512 * (j + 1))
                            nc.tensor.matmul(ps2[:, col], w2[:, :, cs],
                                             h18[:, :, sl],
                                             start=True, stop=True,
                                             perf_mode=DR)
                        t = kpool.tile([128, 1024], BF16, tag="h2",
                                       name=f"h2{cn}{fc}{ntp}", bufs=6,
                                       uniquify=True)
                        relu_out(t[:], ps2[:], b2[fc])
                        h2t[(fc, ntp)] = t
                    yield

                if pre == "k":
                    q, hp = h // 4, h % 4
                    # 4-way column-packed: col tile s covers keys quarter s
                    psk = big_tile(f"psk{cn}")
                    # stationary-major order: 4 col-packed w3a matmuls, then
                    # 4 col-packed w3b (shared stationary, concurrent cols)
                    for s in range(4):
                        ntp, j = s // 2, s % 2
                        col = slice(512 * j, 512 * (j + 1))
                        nc.tensor.matmul(psk[32 * s:32 * (s + 1), 0:512],
                                         w3a, h2t[(0, ntp)][:, col],
                                         start=True, stop=False,
                                         tile_position=(0, 32 * s))
                    for s in range(4):
                        ntp, j = s // 2, s % 2
                        col = slice(512 * j, 512 * (j + 1))
                        nc.tensor.matmul(psk[32 * s:32 * (s + 1), 0:512],
                                         w3b, h2t[(1, ntp)][:, col],
                                         start=False, stop=True,
                                         tile_position=(0, 32 * s))
                    # kb3 cancels in softmax (constant per (p,h)) -> no bias
                    for s in range(4):
                        dst = kt[l][q][32 * hp:32 * (hp + 1),
                                       512 * s:512 * (s + 1)]
                        copy_out(dst, psk[32 * s:32 * (s + 1), 0:512])
                else:
                    psv = big_tile(f"psv{cn}")
                    for svg in range(16):
                        ntp, w = svg // 8, svg % 8
                        j, c = w // 4, w % 4
                        vsl = slice(512 * j + 128 * c,
                                    512 * j + 128 * (c + 1))
                        osl = slice(32 * svg, 32 * (svg + 1))
                        nc.tensor.matmul(
                            psv[:, osl], h2t[(0, ntp)][:, vsl], w3a,
                            start=(svg == 0), stop=False)
                        nc.tensor.matmul(
                            psv[:, osl], h2t[(1, ntp)][:, vsl], w3b,
                            start=False, stop=(svg == 15))
                    vt = vals[l][h]
                    nc.vector.tensor_copy(
                        vt[:, :, 0:AD],
                        psv[:, 0:512].rearrange("p (s d) -> p s d", d=AD))
                    nc.vector.memset(vt[:, :, AD:AD + 1], 1.0)
                yield

            def split_bf(src_tiles, tagp, need_lo=True):
                """f32 [128,P] tiles -> (hi bf16, lo bf16) tiles."""
                his, los = [], []
                for q, s in enumerate(src_tiles):
                    hi = kpool.tile([128, P], BF16, tag=f"{tagp}h{q}",
                                    name=f"{tagp}h{q}", bufs=2, uniquify=True)
                    nc.vector.tensor_copy(hi[:], s[:])
                    his.append(hi)
                    if need_lo:
                        lo = kpool.tile([128, P], BF16, tag=f"{tagp}l{q}",
                                        name=f"{tagp}l{q}", bufs=2,
                                        uniquify=True)
                        nc.vector.tensor_tensor(lo[:], s[:], hi[:],
                                                ALU.subtract)
                        los.append(lo)
                return his, los

            def layer_norm(xq, pfx, l, nm, box):
                """T-layout LN over 256 features (generator; appends two
                f32 tiles to box)."""
                xh, _ = split_bf(xq, "lnx", need_lo=False)
                psum2 = big_tile(f"lnsums{nm}")
                pst = psum2[0:1, 0:512]
                psq = psum2[0:1, 512:1024]
                nc.tensor.matmul(pst, ones_c128b[:], xh[0][:],
                                 start=True, stop=False)
                nc.tensor.matmul(pst, ones_c128b[:], xh[1][:],
                                 start=False, stop=True)
                sq = [kpool.tile([128, P], BF16, tag=f"lnsq{q}",
                                 name=f"lnsq{nm}{q}", bufs=1)
                      for q in range(2)]
                for q in range(2):
                    nc.vector.tensor_tensor(sq[q][:], xh[q][:], xh[q][:],
                                            ALU.mult)
                nc.tensor.matmul(psq, ones_c128b[:], sq[0][:],
                                 start=True, stop=False)
                nc.tensor.matmul(psq, ones_c128b[:], sq[1][:],
                                 start=False, stop=True)
                yield
                mu = kpool.tile([1, P], mybir.dt.float32, tag="lnmu", bufs=1,
                                name=f"lnmu{nm}")
                nc.scalar.mul(mu[:], pst, 1.0 / D)
                m2 = kpool.tile([1, P], mybir.dt.float32, tag="lnm2", bufs=1,
                                name=f"lnm2{nm}")
                nc.vector.tensor_tensor(m2[:], mu[:], mu[:], ALU.mult)
                var = kpool.tile([1, P], mybir.dt.float32, tag="lnvar",
                                 bufs=1, name=f"lnvar{nm}")
                nc.vector.scalar_tensor_tensor(
                    var[:], psq, 1.0 / D, m2[:], ALU.mult, ALU.subtract)
                # rstd = exp(-0.5*ln(var+eps)) -- Ln/Exp share one ACT table
                # set (no Sqrt-set reload, no slow DVE reciprocal)
                lnv = kpool.tile([1, P], mybir.dt.float32, tag="lnsd", bufs=1,
                                 name=f"lnsd{nm}")
                nc.scalar.activation(lnv[:], var[:], AF.Ln, bias=eps_t[:])
                # rstd and -mu*rstd written directly as bf16 broadcast
                # operands (f32 matmuls are half-rate with slow LDWs)
                rsb = kpool.tile([1, 2, P], BF16, tag="lnrsb", bufs=1,
                                 name=f"lnrsb{nm}")
                nc.scalar.activation(rsb[:, 0, :], lnv[:], AF.Exp, scale=-0.5)
                nc.vector.scalar_tensor_tensor(
                    rsb[:, 1, :], mu[:], -1.0, rsb[:, 0, :],
                    ALU.mult, ALU.mult)
                yield
                # broadcast A=rstd, B=-mu*rstd to 128 partitions
                psab = big_tile(f"lnAB{nm}")
                nc.tensor.matmul(psab[:, 0:512], ones_rb16[:],
                                 rsb[:, 0, :], start=True, stop=True)
                nc.tensor.matmul(psab[:, 512:1024], ones_rb16[:],
                                 rsb[:, 1, :], start=True, stop=True)
                for q in range(2):
                    g = lnw[(pfx + "g", l, q)]
                    bb = lnw[(pfx + "b", l, q)]
                    t1 = kpool.tile([128, P], mybir.dt.float32, tag=f"lnt{q}",
                                    name=f"lnt{nm}{q}", bufs=1)
                    nc.vector.tensor_tensor(t1[:], xq[q][:], psab[:, 0:512],
                                            ALU.mult)
                    nc.vector.tensor_tensor(t1[:], t1[:], psab[:, 512:1024],
                                            ALU.add)
                    o = kpool.tile([128, P], mybir.dt.float32, tag=f"attv{q}",
                                   name=f"ln_out{nm}{q}", bufs=2)
                    nc.vector.tensor_scalar(o[:], t1[:], g[:], bb[:],
                                            ALU.mult, ALU.add)
                    box.append(o)
                yield

            def attention(l, attv, out_box, av_lag=0):
                """Count-matrix softmax attention (generator; yields per vc).
                4-way row-packed QK per quad; appends LN output to out_box.
                av_lag delays AV emission so early steps need no vals yet."""
                qt_h, _ = split_bf(attv, "qt", need_lo=False)
                numer = [kpool.tile([128, P], mybir.dt.float32, tag=f"num{q}",
                                    name=f"numer{l}{q}", bufs=1)
                         for q in range(2)]
                xres = []

                def ep_tail(q, rds):
                    """recip broadcast + residual for quad q -- deferred so
                    PE work can be emitted in between.  The four per-head
                    recip rows broadcast via col-packed rank-1 matmuls."""
                    psrb = big_tile(f"psrb{l}{q}")
                    for i in range(4):
                        nc.tensor.matmul(psrb[32 * i:32 * (i + 1), 0:512],
                                         ones_rb16[:, 0:32], rds[i][:],
                                         start=True, stop=True,
                                         tile_position=(0, 32 * i))
                    t1 = kpool.tile([128, P], mybir.dt.float32,
                                    tag=f"xres{q}", name=f"xres{l}{q}",
                                    bufs=1)
                    nc.vector.tensor_tensor(t1[:], numer[q][:],
                                            psrb[:, 0:512], ALU.mult)
                    # + attv (residual) + vb3 (value-bias; softmax wts sum=1)
                    nc.vector.scalar_tensor_tensor(
                        t1[:], t1[:], vb3t[(l, q)][:], attv[q][:],
                        ALU.add, ALU.add)
                    xres.append(t1)

                def emit_av(q, psA, vc, es):
                    for g in range(2):
                        h0 = 4 * q + 2 * g
                        nc.tensor.matmul(
                            psA[g][0:AD + 1, :],
                            vals[l][h0][:, vc, :],
                            es[g][:, 0:512],
                            start=(vc == 0), stop=(vc == 15))
                        nc.tensor.matmul(
                            psA[g][64:64 + AD + 1, :],
                            vals[l][h0 + 1][:, vc, :],
                            es[g][:, 512:1024],
                            start=(vc == 0), stop=(vc == 15),
                            tile_position=(0, 64))

                dn4s = [None, None]
                for q in range(2):
                    psA = [psA_tile(f"psA{l}{q}{g}") for g in range(2)]
                    avq = []
                    for vc in range(16):
                        # all 4 QKs back-to-back: rows 0/32/64/96 pack
                        # concurrently in the PE sub-arrays
                        psst = [big_tile(f"pss{l}{q}{g}{vc}")
                                for g in range(2)]
                        for g in range(2):
                            for hp in (2 * g, 2 * g + 1):
                                bb = 32 * hp
                                tp = (bb, 0) if bb >= 64 else None
                                col = slice(512 * (hp % 2),
                                            512 * (hp % 2 + 1))
                                nc.tensor.matmul(
                                    psst[g][:, col],
                                    kt[l][q][bb:bb + 32,
                                             128 * vc:128 * (vc + 1)],
                                    qt_h[q][bb:bb + 32, :],
                                    start=True, stop=True, tile_position=tp)
                        es = []
                        for g in range(2):
                            e = kpool.tile([128, 1024], BF16, tag="ebuf",
                                           name=f"e{l}{q}{g}{vc}", bufs=3,
                                           uniquify=True)
                            nc.scalar.activation(e[:], psst[g][:], AF.Exp,
                                                 scale=SCALE)
                            ce = kpool.tile([128, 1024], BF16, tag="cebuf",
                                            name=f"ce{l}{q}{g}{vc}", bufs=12,
                                            uniquify=True)
                            nc.vector.tensor_tensor(
                                ce[:], e[:],
                                ct[:, vc:vc + 1, :].broadcast_to(
                                    [128, 2, P]),
                                ALU.mult)
                            es.append(ce)
                        avq.append((vc, es))
                        if len(avq) > av_lag:
                            emit_av(q, psA, *avq.pop(0))
                        yield
                        if q == 1 and vc == 2:
                            ep_tail(0, dn4s[0])
                    for vc_, es_ in avq:
                        emit_av(q, psA, vc_, es_)
                    # psA evacuation inline (frees the AV psum ring for
                    # the next quad): numerators to SBUF, denominators to
                    # per-head recip rows exp(-ln d) straight from PSUM
                    rds = []
                    for g in range(2):
                        b0, b1r = 32 * (2 * g), 32 * (2 * g + 1)
                        copy_out(numer[q][b0:b0 + 32, :], psA[g][0:32, :])
                        copy_out(numer[q][b1r:b1r + 32, :], psA[g][64:96, :])
                        for hl, prow in ((2 * g, 32), (2 * g + 1, 96)):
                            lnd = kpool.tile([1, P], mybir.dt.float32,
                                             tag="lnd1", bufs=3,
                                             name=f"lnd{l}{q}{hl}",
                                             uniquify=True)
                            nc.scalar.activation(lnd[:],
                                                 psA[g][prow:prow + 1, :],
                                                 AF.Ln)
                            rd = kpool.tile([1, P], BF16, tag="rd1", bufs=8,
                                            name=f"rd{l}{q}{hl}",
                                            uniquify=True)
                            nc.scalar.activation(rd[:], lnd[:], AF.Exp,
                                                 scale=-1.0)
                            rds.append(rd)
                    dn4s[q] = rds
                yield
                ep_tail(1, dn4s[1])
                yield
                yield from layer_norm(xres, "ln1", l, f"ln1_{l}", out_box)

            def ff_block(l, attv, out_box):
                """Feed-forward block (generator; yields between stages)."""
                av16 = []
                for kc in range(2):
                    t = kpool.tile([128, P], FP16, tag=f"ff16{kc}",
                                   name=f"ffav{l}{kc}", bufs=2)
                    nc.vector.tensor_copy(t[:], attv[kc][:])
                    av16.append(t)
                yield
                hh_t = []
                for fc in range(2):
                    psf = big_tile(f"psff1{l}{fc}")
                    for kc in range(2):
                        nc.tensor.matmul(
                            psf[:, 0:512],
                            fw1[l][kc][:, 128 * fc:128 * (fc + 1)],
                            av16[kc][:], start=(kc == 0), stop=(kc == 1))
                    th = kpool.tile([128, P], FP16, tag=f"ffhh{fc}",
                                    name=f"ffhh{l}{fc}", bufs=2)
                    relu_out(th[:], psf[:, 0:512], ffb[("b1", l, fc)][:])
                    hh_t.append(th)
                    yield
                xres2 = []
                for fc in range(2):
                    psf2 = big_tile(f"psff2{l}{fc}")
                    for kc in range(2):
                        nc.tensor.matmul(
                            psf2[:, 0:512],
                            fw2[l][kc][:, 128 * fc:128 * (fc + 1)],
                            hh_t[kc][:], start=(kc == 0), stop=(kc == 1))
                    t2 = kpool.tile([128, P], mybir.dt.float32,
                                    tag=f"xres{fc}", name=f"xr2{l}{fc}",
                                    bufs=1)
                    nc.vector.scalar_tensor_tensor(
                        t2[:], psf2[:, 0:512], ffb[("b2", l, fc)][:],
                        attv[fc][:], ALU.add, ALU.add)
                    xres2.append(t2)
                    yield
                yield from layer_norm(xres2, "ln2", l, f"ln2_{l}", out_box)

            # =============================================================
            # Emit program
            # =============================================================
            # ds projection: attv0 = cur @ ds_W + ds_b   (T-layout out);
            # deferred into the v0 phase to keep startup DMAs off the
            # first chains' critical path
            dsw16 = [cpool.tile([128, D], FP16, name=f"dsw16{kc}")
                     for kc in range(2)]
            dsb = [cpool.tile([128, 1], F32, name=f"dsb{q}")
                   for q in range(2)]
            attv = []

            def emit_ds():
                for kc in range(2):
                    nc.sync.dma_start(dsw16[kc][:],
                                      dsw16d[128 * kc:128 * (kc + 1), :])
                    nc.sync.dma_start(cur16[kc][:],
                                      cur16d[128 * kc:128 * (kc + 1), :])
                for q in range(2):
                    nc.sync.dma_start(dsb[q][:], dsbd[q, :, :])

            def emit_ds2():
                for q in range(2):
                    psd = big_tile(f"psds{q}")
                    for kc in range(2):
                        nc.tensor.matmul(
                            psd[:, 0:512],
                            dsw16[kc][:, 128 * q:128 * (q + 1)],
                            cur16[kc][:], start=(kc == 0), stop=(kc == 1))
                    o = kpool.tile([128, P], mybir.dt.float32,
                                   tag=f"attv{q}", name=f"attv0{q}", bufs=2)
                    nc.vector.tensor_scalar(o[:], psd[:, 0:512], dsb[q][:],
                                            None, ALU.add)
                    attv.append(o)

            # --- pipelined emission ------------------------------------
            def drain(g):
                for _ in g:
                    pass

            def weave_gen(gens, after=()):
                """Cross-chain pipeline (generator): chain c's mm3 tail is
                emitted right after chain c+1's first mm1 wave.  `after` is
                a sequence of (chain_idx, fn) emission hooks."""
                hooks = dict(after)
                tail = None
                for idx, g in enumerate(gens):
                    next(g)
                    yield
                    if tail is not None:
                        for _ in tail:
                            yield
                    for _ in range(3):
                        next(g)
                        yield
                    tail = g
                    if idx in hooks:
                        hooks[idx]()
                if tail is not None:
                    for _ in tail:
                        yield

            def par2(agen, asteps, cgen, ratio):
                """Advance agen by asteps yields, interleaving ~ratio chain
                steps per attention step."""
                for _ in range(asteps):
                    try:
                        next(agen)
                    except StopIteration:
                        break
                    if cgen is not None:
                        for _ in range(ratio):
                            try:
                                next(cgen)
                            except StopIteration:
                                cgen = None
                                break

            k0 = [kv_chain(0, hh, "k") for hh in range(H)]
            v0 = [kv_chain(0, hh, "v") for hh in range(H)]
            k1 = [kv_chain(1, hh, "k") for hh in range(H)]
            v1 = [kv_chain(1, hh, "v") for hh in range(H)]

            drain(weave_gen(k0[0:4], after=(
                (1, lambda: (hoists[0](), emit_ds())), (2, hoists[1]),
                (3, hoists[2]))))
            # attn0-q0's QK/exp/ce need only kt0[q0] + attv; with av_lag=6
            # its first steps overlap the v0 chains that produce vals0.
            c_head = weave_gen(v0[0:4], after=(
                (0, hoists[3]), (1, lambda: (hoists[4](), emit_ds2()))))
            for _ in range(12):
                next(c_head)
            box0 = []
            a0 = attention(0, attv, box0, av_lag=5)
            par2(a0, 6, c_head, ratio=2)
            drain(c_head)
            # q1 needs k0[4:]+v0[4:] -> fully emitted within q0's steps.
            c1a = weave_gen(k0[4:8] + v0[4:8],
                            after=((3, hoists[5]), (6, hoists[6])))
            par2(a0, 10, c1a, ratio=4)
            drain(c1a)
            # q1 + tail interleave with k1 (attn1-q0 needs all of kt1
            # emitted before attention(1) starts).
            c1b = weave_gen(k1)
            par2(a0, 21, c1b, ratio=2)
            drain(a0)
            drain(c1b)
            attv = box0
            # v1 chains fill the PE-idle ff0/attn1 windows: v1[0:4] must be
            # fully emitted before attn1's first AV (quad0), v1[4:8] before
            # its quad-1 AVs.
            c1ca = weave_gen(v1[0:4])
            box_f0 = []
            f0 = ff_block(0, attv, box_f0)
            par2(f0, 8, c1ca, ratio=3)
            drain(f0)
            drain(c1ca)
            attv = box_f0

            box1 = []
            a1 = attention(1, attv, box1, av_lag=4)
            c1cb = weave_gen(v1[4:8])
            par2(a1, 37, c1cb, ratio=2)
            drain(c1cb)
            drain(a1)
            attv = box1
            box_f1 = []
            drain(ff_block(1, attv, box_f1))
            attv = box_f1

            # ---- decoder ----------------------------------------------
            de16 = []
            for kc in range(2):
                t = kpool.tile([128, P], FP16, tag=f"de16{kc}",
                               name=f"de16{kc}", bufs=1)
                nc.vector.tensor_copy(t[:], attv[kc][:])
                de16.append(t)
            d1 = []
            for fc in range(2):
                psd1 = big_tile(f"psde1{fc}")
                for kc in range(2):
                    nc.tensor.matmul(
                        psd1[:, 0:512],
                        dw1[kc][:, 128 * fc:128 * (fc + 1)],
                        de16[kc][:], start=(kc == 0), stop=(kc == 1))
                th = kpool.tile([128, P], FP16, tag=f"d1h{fc}",
                                name=f"d1h{fc}", bufs=2)
                relu_out(th[:], psd1[:, 0:512], deb1[fc][:])
                d1.append(th)
            d2 = []
            for fc in range(2):
                psd2 = big_tile(f"psde2{fc}")
                for kc in range(2):
                    nc.tensor.matmul(
                        psd2[:, 0:512],
                        dw2[kc][:, 128 * fc:128 * (fc + 1)],
                        d1[kc][:], start=(kc == 0), stop=(kc == 1))
                th = kpool.tile([128, P], FP16, tag=f"d2h{fc}",
                                name=f"d2h{fc}", bufs=2)
                relu_out(th[:], psd2[:, 0:512], deb2[fc][:])
                d2.append(th)

            # logits row-major [p, R] per 128-p chunk + loss.  Targets are
            # host-precomputed one-hots; logits are O(10) so exp needs no
            # max-shift in f32.
            oht = kpool.tile([128, 4, R], mybir.dt.float32, tag="oht",
                             name="oht", bufs=1)
            nc.sync.dma_start(oht[:], ohtd[:])
            t4 = kpool.tile([128, 4], mybir.dt.float32, tag="t4",
                            name="t4", bufs=1)

            def logits_tail(pc, psl):
                escr = kpool.tile([128, R], mybir.dt.float32, tag="escr",
                                  bufs=2, name=f"escr{pc}")
                se = kpool.tile([128, 1], mybir.dt.float32, tag="se",
                                name=f"se{pc}", bufs=2)
                nc.scalar.activation(escr[:], psl[:, 0:R], AF.Exp,
                                     accum_out=se[:])
                ls = kpool.tile([128, 1], mybir.dt.float32, tag="ls",
                                name=f"ls{pc}", bufs=2)
                nc.scalar.activation(ls[:], se[:], AF.Ln)
                scr2 = kpool.tile([128, R], mybir.dt.float32, tag="scr2",
                                  bufs=2, name=f"scr2{pc}")
                pk = kpool.tile([128, 1], mybir.dt.float32, tag="pk",
                                name=f"pk{pc}", bufs=2)
                nc.vector.scalar_tensor_tensor(
                    scr2[:], psl[:, 0:R], 1.0, oht[:, pc, :],
                    ALU.mult, ALU.mult, accum_out=pk[:])
                nc.vector.tensor_tensor(t4[:, pc:pc + 1], pk[:], ls[:],
                                        ALU.subtract)

            prev_log = None
            for pc in range(4):
                psl = big_tile(f"pslog{pc}")
                psl_sl = slice(128 * pc, 128 * (pc + 1))
                # bias row (f32 rank-1), then the two K-chunks
                nc.tensor.matmul(psl[:, 0:R], ones_rbf[:], db3f[:],
                                 start=True, stop=False)
                for kc in range(2):
                    nc.tensor.matmul(psl[:, 0:R], d2[kc][:, psl_sl],
                                     dw3[kc][:],
                                     start=False, stop=(kc == 1))
                if prev_log is not None:
                    logits_tail(*prev_log)
                prev_log = (pc, psl)
            logits_tail(*prev_log)
            pspr = big_tile("pspr")
            nc.tensor.matmul(pspr[0:1, 0:4], ones_c128f[:], t4[:],
                             start=True, stop=True)
            pr4 = kpool.tile([1, 4], mybir.dt.float32, tag="pr4",
                             name="pr4", bufs=1)
            nc.vector.tensor_copy(pr4[:], pspr[0:1, 0:4])
            s1 = kpool.tile([1, 1], mybir.dt.float32, tag="s1",
                            name="s1", bufs=1)
            nc.vector.tensor_reduce(s1[:], pr4[:], mybir.AxisListType.X,
                                    ALU.add)
            outt = kpool.tile([1, 1], mybir.dt.float32, tag="outt",
                              name="outt", bufs=1)
            nc.scalar.activation(outt[:], s1[:], AF.Identity,
                                 bias=nlogr_t[:], scale=-1.0)
            nc.sync.dma_start(out_d[:], outt[:])

    return nc


F8 = getattr(ml_dtypes, "float8_e4m3", ml_dtypes.float8_e4m3fn)


def _kpack(w, dt=None):
    """[256, F] -> K-halves packed [128, 2, F]."""
    w = np.asarray(w, np.float32)
    return np.ascontiguousarray(
        w.reshape(2, 128, w.shape[-1]).transpose(1, 0, 2)).astype(dt or F8)


def _maybe_enable_trace():
    """Optional NTFF profiling under axon (KERNEL_TRACE=1); best-effort."""
    try:
        import sys
        import types

        import antenv

        if "antenv.axon_hooks" not in sys.modules:
            mod = types.ModuleType("antenv.axon_hooks")
            mod._hook = None
            mod.set_axon_ntff_profile_hook = lambda h: setattr(mod, "_hook", h)
            mod.get_axon_ntff_profile_hook = lambda: mod._hook
            sys.modules["antenv.axon_hooks"] = mod
            antenv.axon_hooks = mod
            from trn_agent_boot.trn_boot import _ntff_profile_via_ctypes

            mod._hook = _ntff_profile_via_ctypes("/opt/axon/libaxon_pjrt.so")
        import concourse.bass_utils as _bu

        _bu.upload_artifacts = lambda tmpdir: f"file://{tmpdir}"
        return True
    except Exception:
        return False


LAST_RESULT = {}


def _enable_ldw_opt():
    """Turn on walrus LDWEIGHTS dedup (consecutive matmuls sharing a
    stationary tile skip the reload) for our own compile invocation."""
    import concourse.bass_utils as _bu

    if getattr(_bu, "_ldw_opt_patched", False):
        return
    orig = _bu.run_command

    # ldw-opt errors out on tile_position LDWs ("not compatible with LDW
    # optimization") -- left disabled.
    _ = orig
    _bu._ldw_opt_patched = True


def kernel(**inputs):
    from concourse.bass_utils import run_bass_kernel_spmd
    _enable_ldw_opt()

    if "nc" not in _BUILT:
        _BUILT["nc"] = _build()
    nc = _BUILT["nc"]

    f32 = lambda a: np.ascontiguousarray(np.asarray(a, np.float32))
    bf = lambda a: np.ascontiguousarray(np.asarray(a, np.float32)).astype(BF)
    f16 = lambda a: np.ascontiguousarray(
        np.asarray(a, np.float32)).astype(np.float16)

    enc = f32(inputs["encoded"])                      # [B,V,I]
    tu = f32(inputs["true_u"])                        # [B,V,1]
    mask = f32(inputs["attn_mask"])                   # [P,N]
    pp_ = np.asarray(inputs["pred_points"]).astype(np.int64)
    ni = np.asarray(inputs["neighbor_index"]).astype(np.int64)

    # count matrix C[p, v]
    C = np.zeros((P, V), np.float32)
    np.add.at(C, (np.repeat(np.arange(P), N), ni.ravel()),
              np.exp(-SCALE * mask).ravel().astype(np.float32))
    ctm = np.ascontiguousarray(C.T).astype(BF)        # [V, P]

    shared = {"ctm": ctm}
    for pre in ("k", "v"):
        w1 = f32(inputs[pre + "W1"])                  # [L,H,257,256]
        w2 = f32(inputs[pre + "W2"])
        shared[pre + "w12"] = np.stack(
            [np.stack([np.concatenate(
                [_kpack(w1[l, h, 0:256]), _kpack(w2[l, h])], axis=1)
                for h in range(H)]) for l in range(L)])
        # u rank-1 row replicated 4x for 4-way row packing
        shared[pre + "1c"] = np.ascontiguousarray(np.broadcast_to(
            w1[:, :, 256:257, :], (L, H, 4, M))).astype(BF)
        w3 = f32(inputs[pre + "W3"])                  # [L,H,256,32]
        shared[pre + "w3"] = np.concatenate(
            [w3[:, :, 0:128, :], w3[:, :, 128:256, :]], axis=3).astype(BF)
        b1 = f32(inputs[pre + "b1"]).reshape(L, H, 2, 128)
        b2 = f32(inputs[pre + "b2"]).reshape(L, H, 2, 128)
        shared[pre + "bb"] = np.ascontiguousarray(
            np.stack([b1[:, :, 0], b1[:, :, 1], b2[:, :, 0], b2[:, :, 1]],
                     axis=3))
    # rename to match din names
    shared["kw1c"] = shared.pop("k1c")
    shared["vw1c"] = shared.pop("v1c")
    # vb3 folded into residual (softmax weights sum to 1); kb3 cancels.
    shared["vb3qd"] = np.ascontiguousarray(
        f32(inputs["vb3"]).reshape(L, 2, 128)[:, :, :, None])

    shared["dsw16d"] = f16(inputs["ds_W"])
    shared["dsbd"] = f32(inputs["ds_b"]).reshape(2, 128, 1)
    shared["ffw1d"] = f16(inputs["ff_W1"])
    shared["ffw2d"] = f16(inputs["ff_W2"])
    shared["ffb1d"] = f32(inputs["ff_b1"]).reshape(L, 2, 128, 1)
    shared["ffb2d"] = f32(inputs["ff_b2"]).reshape(L, 2, 128, 1)
    shared["ln1gd"] = f32(inputs["ln1_g"]).reshape(L, 2, 128, 1)
    shared["ln1bd"] = f32(inputs["ln1_b"]).reshape(L, 2, 128, 1)
    shared["ln2gd"] = f32(inputs["ln2_g"]).reshape(L, 2, 128, 1)
    shared["ln2bd"] = f32(inputs["ln2_b"]).reshape(L, 2, 128, 1)
    shared["dew1d"] = f16(inputs["de_W1"])
    shared["dew2d"] = f16(inputs["de_W2"])
    shared["dew3d"] = f16(inputs["de_W3"])
    shared["deb1d"] = f32(inputs["de_b1"]).reshape(2, 128, 1)
    shared["deb2d"] = f32(inputs["de_b2"]).reshape(2, 128, 1)
    shared["deb3d"] = f32(inputs["de_b3"]).reshape(1, R)

    oh4f = np.zeros((4, 128), np.float32)
    for i in range(4):
        oh4f[i, 32 * i:32 * (i + 1)] = 1.0
    shared["oh8d"] = oh4f.astype(BF)

    in_maps = []
    for b in range(B):
        merged = np.concatenate([enc[b], tu[b]], axis=1)  # [V, 257]
        mt = np.ascontiguousarray(merged.T)               # [257, V]
        cur = enc[b][pp_, :]                              # [P, I]
        curt = np.ascontiguousarray(cur.T)                # [I, P]
        m = dict(shared)
        m["xtd"] = _kpack(mt[0:256])
        m["xt2"] = np.ascontiguousarray(
            np.broadcast_to(mt[256:257], (4, V))).astype(BF)
        m["cur16d"] = curt.astype(np.float16)
        m["updc"] = tu[b][pp_, :]                          # [P,1] f32
        tgt = np.clip(np.floor(tu[b][pp_, 0] * R).astype(np.int64), 0, R - 1)
        oht = np.zeros((P, R), np.float32)
        oht[np.arange(P), tgt] = 1.0
        # oht[p_local, pc, r]
        m["ohtd"] = np.ascontiguousarray(
            oht.reshape(4, 128, R).transpose(1, 0, 2))
        in_maps.append(m)

    trace = os.environ.get("KERNEL_TRACE") == "1" and _maybe_enable_trace()
    res = run_bass_kernel_spmd(
        nc, in_maps, core_ids=list(range(B)), trace=trace,
        trace_cores=list(range(B)) if trace else None)
    LAST_RESULT["res"] = res
    if trace and res.exec_time_ns is not None:
        print(f"HW exec time: {res.exec_time_ns} ns "
              f"(mean {res.mean_exec_time_ns} ns, "
              f"slowest core {res.max_exec_time_core_id})")
    out = np.array([res.results[b]["out"][0, 0] for b in range(B)], np.float32)
    return out
